# revision 1
# baseline (speedup 1.0000x reference)
"""Trainium2 Bass kernel for nn_NodeDetector (masked-node GATv2 ensemble).

Algorithm: the reference vmaps a full 2-layer GATv2 over 256 "masked node"
variants, but variant v differs from the shared base computation in exactly
one input row (row v).  We compute the base graph once and apply sparse
incremental updates per variant:

  phase 0  dense projections -> XL/XR (base rows) and XLs/XRs (masked rows)
  phase 1  base GAT layer 1: per-dst softmax sums (num1/den1) + g1_base
  (a)      per variant v: "light" g1 updates at out-neighbors d of v
           (only edges v->d changed: closed-form num/den delta)
  (b)      per variant v: full recompute of g1 at node v
  (d)      layer 2 at dst v only: gather xl2 of in-neighbors (base / self /
           rare light rows), one softmax, project + tanh.

Attention softmaxes skip the per-dst max subtraction (mathematically
identical; logits are O(10) so fp32 exp is safe).  All gathers use
host-built index tables (edge_index is host data) via gpsimd indirect DMA.
Work is sharded 32 variants per core across 8 cores; phases 0/1 are
replicated per core.  No collectives.
"""

import numpy as np

import concourse.bass as bass
import concourse.mybir as mybir
import concourse.tile as tile
from concourse import bacc
from concourse.bass_utils import run_bass_kernel_spmd
from concourse.masks import make_identity

F32 = mybir.dt.float32
I32 = mybir.dt.int32
AF = mybir.ActivationFunctionType
OP = mybir.AluOpType
AX = mybir.AxisListType

N = 256          # nodes / variants
F = 128          # NUM_HEAD * C2
C2 = 64
NH = 2
NCORES = 8
VPC = N // NCORES   # variants per core = 32
JC = 4              # partition-split of each dst's in-edge list
NEG = 0.2           # leaky relu slope


# --------------------------------------------------------------------------
# Host-side table construction
# --------------------------------------------------------------------------

def _build_tables(edge_index):
    src = edge_index[0].astype(np.int64)
    dst = edge_index[1].astype(np.int64)
    E = src.shape[0]

    in_edges = [[] for _ in range(N)]
    for e in range(E):
        in_edges[dst[e]].append(e)
    max_in = max(len(l) for l in in_edges)
    SLOTS = -(-max_in // JC)            # in-edge slots per jc row
    out_by_src = [[] for _ in range(N)]
    for e in range(E):
        if dst[e] != src[e]:
            out_by_src[src[e]].append(int(dst[e]))
    light = []
    for v in range(N):
        cnt = {}
        for d in out_by_src[v]:
            cnt[d] = cnt.get(d, 0) + 1
        light.append(sorted(cnt.items()))
    max_light = max(len(l) for l in light)
    K2 = 4 * (-(-max_light // 4))       # light slots per variant, mult of 4
    KA = K2 * VPC // 128                # light slots per partition

    def wrap16(flat):
        """int16 idx layout for dma_gather: value for flat position i lives
        at [i % 16, i // 16], tiled to 128 partitions."""
        flat = np.asarray(flat)
        num = flat.shape[0]
        A = np.zeros((16, num // 16), np.int16)
        A[np.arange(num) % 16, np.arange(num) // 16] = flat.astype(np.int16)
        return np.ascontiguousarray(np.tile(A, (8, 1)))

    def wrapPK(idx_pk):
        """[128, K] logical idx (out[p, k] = tab[idx_pk[p,k]]) -> wrapped."""
        return wrap16(idx_pk.T.reshape(-1))

    shared = {}
    IDX_P1 = np.zeros((N * JC, SLOTS), np.int32)
    MSK_P1 = np.zeros((N * JC, SLOTS), np.float32)
    IDXD_P1 = np.zeros((N * JC, 1), np.int32)
    for d in range(N):
        el = in_edges[d]
        for jc in range(JC):
            g = d * JC + jc
            IDXD_P1[g, 0] = d
            for s in range(SLOTS):
                k = jc * SLOTS + s
                if k < len(el):
                    IDX_P1[g, s] = src[el[k]]
                    MSK_P1[g, s] = 1.0
    # wrapped gather indices per half: out[p, t*SLOTS+s] = IDX_P1[512h+128t+p, s]
    IDX_P1W = np.zeros((2, 128, 4 * SLOTS * 128 // 16), np.int16)
    for h in range(2):
        pk = np.zeros((128, 4 * SLOTS), np.int64)
        for p in range(128):
            for t in range(4):
                pk[p, t * SLOTS:(t + 1) * SLOTS] = IDX_P1[512 * h + 128 * t + p]
        IDX_P1W[h] = wrapPK(pk)
    shared["IDX_P1W"] = IDX_P1W
    shared["MSK_P1"] = MSK_P1
    shared["IDXD_P1"] = IDXD_P1

    # combine matrices, [128, 4, 128]: input tile t', partial row p -> col
    CMB4 = np.zeros((128, 4, 128), np.float32)
    for tp in range(4):
        for p in range(128):
            CMB4[p, tp, 32 * tp + p // 4] = 1.0
    shared["CMB4"] = CMB4

    percore = []
    for c in range(NCORES):
        t = {}
        V = list(range(c * VPC, (c + 1) * VPC))
        IDX_A_T1 = np.zeros((128, KA), np.int32)
        IDX_A_V = np.zeros((128, KA), np.int32)
        IDX_A_VS = np.zeros((128, KA), np.int32)
        C_A = np.zeros((128, KA), np.float32)
        for r in range(128 * KA):
            vi, slot = divmod(r, K2)
            p, k = divmod(r, KA)
            v = V[vi]
            IDX_A_V[p, k] = v
            IDX_A_VS[p, k] = 256 + v
            if slot < len(light[v]):
                d, cc = light[v][slot]
                IDX_A_T1[p, k] = d
                C_A[p, k] = float(cc)
        t["IDX_A_T1W"] = wrapPK(IDX_A_T1)
        t["IDX_A_VW"] = wrapPK(IDX_A_V)
        t["IDX_A_VSW"] = wrapPK(IDX_A_VS)
        t["C_A"] = C_A

        IDX_B_XL = np.zeros((128, SLOTS), np.int32)
        MSK_B = np.zeros((128, SLOTS), np.float32)
        IDX_B_V = np.zeros((128, 1), np.int32)
        for vi, v in enumerate(V):
            el = in_edges[v]
            for jc in range(JC):
                p = vi * JC + jc
                IDX_B_V[p, 0] = v
                for s in range(SLOTS):
                    k = jc * SLOTS + s
                    if k < len(el):
                        sn = int(src[el[k]])
                        IDX_B_XL[p, s] = 256 + v if sn == v else sn
                        MSK_B[p, s] = 1.0
        t["IDX_B_XLW"] = wrapPK(IDX_B_XL)
        t["MSK_B"] = MSK_B
        t["IDX_B_V"] = IDX_B_V

        IDX_D_T2 = np.zeros((128, SLOTS), np.int32)
        MSK_D = np.zeros((128, SLOTS), np.float32)
        IDX_D_V = np.zeros((128, 1), np.int32)
        IDX_RARE = np.zeros((128, 1), np.int32)
        rare_map = {}
        for vi, v in enumerate(V):
            lpos = {d: i for i, (d, _) in enumerate(light[v])}
            el = in_edges[v]
            for jc in range(JC):
                p = vi * JC + jc
                IDX_D_V[p, 0] = vi
                for s in range(SLOTS):
                    k = jc * SLOTS + s
                    if k < len(el):
                        sn = int(src[el[k]])
                        MSK_D[p, s] = 1.0
                        if sn == v:
                            idx = 256 + vi
                        elif sn in lpos:
                            key = (vi, sn)
                            if key not in rare_map:
                                rs = len(rare_map)
                                assert rs < 128, "rare-row overflow"
                                rare_map[key] = rs
                                IDX_RARE[rs, 0] = vi * K2 + lpos[sn]
                            idx = 288 + rare_map[key]
                        else:
                            idx = sn
                        IDX_D_T2[p, s] = idx
        t["IDX_D_T2W"] = wrapPK(IDX_D_T2)
        t["MSK_D"] = MSK_D
        t["IDX_D_V"] = IDX_D_V
        t["IDX_RARE"] = IDX_RARE
        percore.append(t)

    dims = dict(SLOTS=SLOTS, K2=K2, KA=KA)
    return shared, percore, dims


# --------------------------------------------------------------------------
# Device program
# --------------------------------------------------------------------------

def _build_program(dims, lrelu_act=True, dbg=False):
    SLOTS, K2, KA = dims["SLOTS"], dims["K2"], dims["KA"]

    nc = bacc.Bacc("TRN2", target_bir_lowering=False, debug=False)

    def inp(name, shape, dtype=F32):
        return nc.dram_tensor(name, list(shape), dtype, kind="ExternalInput")

    D = {}
    D["x"] = inp("x", [N, 64])
    D["E_emb"] = inp("E_emb", [N, 64])
    for nm, sh in [("node_proj", [64, 128]), ("emb_proj", [64, 128]),
                   ("conv_w0", [128, 128]), ("conv_w1", [128, 128]),
                   ("conv_b", [128, 1]), ("lin2_w", [128, 64]),
                   ("lin2_b", [64, 1]), ("masked_proj", [64, 64]),
                   ("normal_proj", [64, 64]), ("g1_wl", [64, 128]),
                   ("g1_bl", [128, 1]), ("g1_wr", [64, 128]),
                   ("g1_br", [128, 1]), ("g2_wl", [64, 128]),
                   ("g2_wr", [64, 128]), ("rec_w", [64, 64]),
                   ("rec_b", [64, 1]), ("att1_rep", [128, 128]),
                   ("att2_rep", [128, 128]), ("g1bias_rep", [128, 64]),
                   ("g2bias_rep", [128, 64]), ("blr_rep", [128, 128]),
                   ("CMB4", [128, 4, 128])]:
        D[nm] = inp(nm, sh)
    for nm, sh in [("IDXD_P1", [N * JC, 1]), ("IDX_B_V", [128, 1]),
                   ("IDX_D_V", [128, 1]), ("IDX_RARE", [128, 1])]:
        D[nm] = inp(nm, sh, I32)
    I16 = mybir.dt.int16
    for nm, sh in [("IDX_P1W", [2, 128, 4 * SLOTS * 8]),
                   ("IDX_A_T1W", [128, KA * 8]),
                   ("IDX_A_VW", [128, KA * 8]),
                   ("IDX_A_VSW", [128, KA * 8]),
                   ("IDX_B_XLW", [128, SLOTS * 8]),
                   ("IDX_D_T2W", [128, SLOTS * 8])]:
        D[nm] = inp(nm, sh, I16)
    for nm, sh in [("MSK_P1", [N * JC, SLOTS]), ("C_A", [128, KA]),
                   ("MSK_B", [128, SLOTS]), ("MSK_D", [128, SLOTS])]:
        D[nm] = inp(nm, sh)

    D["out"] = nc.dram_tensor("out", [VPC, 64], F32, kind="ExternalOutput")
    D["XLcat"] = nc.dram_tensor("XLcat", [2 * N, F], F32)
    D["XRtab"] = nc.dram_tensor("XRtab", [N, F], F32)
    D["XRStab"] = nc.dram_tensor("XRStab", [N, F], F32)
    D["T1"] = nc.dram_tensor("T1", [N, 320], F32)
    D["G1L"] = nc.dram_tensor("G1L", [VPC * K2, C2], F32)
    D["T2"] = nc.dram_tensor("T2", [N + VPC + 128, F], F32)
    D["XR2S"] = nc.dram_tensor("XR2S", [VPC, F], F32)

    with tile.TileContext(nc) as tc:
        _trace(nc, tc, D, SLOTS, K2, KA, lrelu_act, dbg)
    nc.compile()
    return nc


def _trace(nc, tc, D, SLOTS, K2, KA, lrelu_act=True, dbg=False):
    import contextlib
    ctx = contextlib.ExitStack()
    with ctx:
        consts = ctx.enter_context(tc.tile_pool(name="consts", bufs=1))
        small = ctx.enter_context(tc.tile_pool(name="small", bufs=1))
        big = ctx.enter_context(tc.tile_pool(name="big", bufs=1))
        psum = ctx.enter_context(tc.tile_pool(name="psum", bufs=4,
                                              space="PSUM"))
        psum_acc = ctx.enter_context(tc.tile_pool(name="psacc", bufs=2,
                                                  space="PSUM"))

        dma = nc.sync.dma_start
        I16 = mybir.dt.int16

        def dgather(out_ap, in_ap, idx_ap, num, elem):
            nc.gpsimd.dma_gather(out_ap=out_ap, in_ap=in_ap, idxs_ap=idx_ap,
                                 num_idxs=num, num_idxs_reg=num,
                                 elem_size=elem, single_packet=False)
        def dbg_dump(name, ap, dtype=F32):
            if not dbg:
                return
            sh = list(ap.shape)
            t_ = nc.dram_tensor("dbg_" + name, sh, dtype,
                                kind="ExternalOutput")
            dma(out=t_[:], in_=ap)
        tt = nc.vector.tensor_tensor
        red = nc.vector.tensor_reduce
        act = nc.scalar.activation
        gather = nc.gpsimd.indirect_dma_start
        IOA = bass.IndirectOffsetOnAxis

        # ---------------- constants ----------------
        ident = consts.tile([128, 128], F32, tag="ident")
        make_identity(nc, ident[:])

        def load(name, shape, dtype=F32):
            t_ = consts.tile(list(shape), dtype, tag="c_" + name)
            dma(out=t_[:], in_=D[name][:])
            return t_

        w_node = load("node_proj", [64, 128])
        w_emb = load("emb_proj", [64, 128])
        w_c0 = load("conv_w0", [128, 128])
        w_c1 = load("conv_w1", [128, 128])
        b_conv = load("conv_b", [128, 1])
        w_lin2 = load("lin2_w", [128, 64])
        b_lin2 = load("lin2_b", [64, 1])
        w_mask = load("masked_proj", [64, 64])
        w_norm = load("normal_proj", [64, 64])
        w_1l = load("g1_wl", [64, 128])
        b_1l = load("g1_bl", [128, 1])
        w_1r = load("g1_wr", [64, 128])
        b_1r = load("g1_br", [128, 1])
        w_2l = load("g2_wl", [64, 128])
        w_2r = load("g2_wr", [64, 128])
        w_rec = load("rec_w", [64, 64])
        b_rec = load("rec_b", [64, 1])
        att1 = load("att1_rep", [128, 128])
        att2 = load("att2_rep", [128, 128])
        g1bias = load("g1bias_rep", [128, 64])
        g2bias = load("g2bias_rep", [128, 64])
        blr = load("blr_rep", [128, 128])
        cmb4 = load("CMB4", [128, 4, 128])

        # ---------------- helpers ----------------
        def ts_mul(out, in0, s):
            nc.vector.tensor_scalar_mul(out=out, in0=in0, scalar1=s)

        def lrelu(flat_ap, nfree, tag):
            if lrelu_act:
                act(out=flat_ap, in_=flat_ap, func=AF.Lrelu, alpha=NEG)
            else:
                t_ = big.tile([128, nfree], F32, tag="lr_" + tag)
                ta = t_[:flat_ap.shape[0], :]
                ts_mul(ta, flat_ap, NEG)
                tt(out=flat_ap, in0=flat_ap, in1=ta, op=OP.max)

        def elu_inplace(x_ap, scratch_pool, nfree, tag):
            xpos = scratch_pool.tile([128, nfree], F32, tag=tag + "_xp")
            nrow = x_ap.shape[0]
            xp = xpos[:nrow, :]
            nc.vector.tensor_scalar_max(out=xp, in0=x_ap, scalar1=0.0)
            nc.vector.tensor_scalar_min(out=x_ap, in0=x_ap, scalar1=0.0)
            act(out=x_ap, in_=x_ap, func=AF.Exp)
            nc.vector.tensor_scalar_add(out=x_ap, in0=x_ap, scalar1=-1.0)
            nc.vector.tensor_add(out=x_ap, in0=x_ap, in1=xp)
            return x_ap

        def head_mean_bias_elu(nd_ap, nrow, bias_rep, tag):
            """nd_ap [nrow, F+NH] = (num|den) -> elu(mean_h(num/den)+bias)."""
            rec = small.tile([128, NH], F32, tag=tag + "_rec")
            nc.vector.reciprocal(out=rec[:nrow, :], in_=nd_ap[:, F:F + NH])
            r0 = small.tile([128, C2], F32, tag=tag + "_r0")
            r1 = small.tile([128, C2], F32, tag=tag + "_r1")
            ts_mul(r0[:nrow, :], nd_ap[:, 0:C2], rec[:nrow, 0:1])
            ts_mul(r1[:nrow, :], nd_ap[:, C2:F], rec[:nrow, 1:2])
            tt(out=r0[:nrow, :], in0=r0[:nrow, :], in1=r1[:nrow, :], op=OP.add)
            ts_mul(r0[:nrow, :], r0[:nrow, :], 0.5)
            tt(out=r0[:nrow, :], in0=r0[:nrow, :], in1=bias_rep[:nrow, :],
               op=OP.add)
            return elu_inplace(r0[:nrow, :], small, C2, tag)

        # ---------------- phase 0 ----------------
        def mm_to_sbuf(lhsT, rhs, M, Nf, tag, bias=None, func=AF.Identity,
                       extra=None):
            out_tile = small.tile([M, Nf], F32, tag=tag)
            ps = psum.tile([128, 256], F32, tag="ps")
            nc.tensor.matmul(ps[:M, :Nf], lhsT, rhs, start=True,
                             stop=extra is None)
            if extra is not None:
                nc.tensor.matmul(ps[:M, :Nf], extra[0], extra[1],
                                 start=False, stop=True)
            if bias is None:
                act(out=out_tile[:], in_=ps[:M, :Nf], func=func)
            else:
                act(out=out_tile[:], in_=ps[:M, :Nf], func=func, bias=bias)
            return out_tile

        xT = small.tile([64, 256], F32, tag="xT")
        eT = small.tile([64, 256], F32, tag="eT")
        for h in range(2):
            for (dname, dstT, tg) in ((("x"), xT, "ldx"), ("E_emb", eT, "lde")):
                tin = small.tile([128, 64], F32, tag="ph0_" + tg)
                dma(out=tin[:], in_=D[dname][128 * h:128 * (h + 1), :])
                pst = psum.tile([64, 128], F32, tag="ps")
                nc.tensor.transpose(pst[:], tin[:], ident[:])
                nc.vector.tensor_copy(out=dstT[:, 128 * h:128 * (h + 1)],
                                      in_=pst[:])

        xpT = mm_to_sbuf(w_node[:], xT[:], 128, 256, "xpT")
        epT = mm_to_sbuf(w_emb[:], eT[:], 128, 256, "epT")
        HbT = mm_to_sbuf(w_c0[:], epT[:], 128, 256, "HbT", bias=b_conv[:],
                         func=AF.Tanh, extra=(w_c1[:], xpT[:]))
        HsT = mm_to_sbuf(w_c0[:], epT[:], 128, 256, "HsT", bias=b_conv[:],
                         func=AF.Tanh)
        MbT = mm_to_sbuf(w_lin2[:], HbT[:], 64, 256, "MbT", bias=b_lin2[:])
        MsT = mm_to_sbuf(w_lin2[:], HsT[:], 64, 256, "MsT", bias=b_lin2[:])
        PbT = mm_to_sbuf(w_norm[:], MbT[:], 64, 256, "PbT")
        PsT = mm_to_sbuf(w_mask[:], MsT[:], 64, 256, "PsT")
        XLT = mm_to_sbuf(w_1l[:], PbT[:], 128, 256, "XLT", bias=b_1l[:])
        XRT = mm_to_sbuf(w_1r[:], PbT[:], 128, 256, "XRT", bias=b_1r[:])
        XLsT = mm_to_sbuf(w_1l[:], PsT[:], 128, 256, "XLsT", bias=b_1l[:])
        XRsT = mm_to_sbuf(w_1r[:], PsT[:], 128, 256, "XRsT", bias=b_1r[:])

        def store_nodemajor(srcT, dram_ap_fn, tag):
            for h in range(2):
                ps = psum.tile([128, 128], F32, tag="ps")
                nc.tensor.transpose(ps[:], srcT[:, 128 * h:128 * (h + 1)],
                                    ident[:])
                sb = small.tile([128, 128], F32, tag="nm_sb_" + tag)
                nc.vector.tensor_copy(out=sb[:], in_=ps[:])
                dma(out=dram_ap_fn(h), in_=sb[:])

        store_nodemajor(XLT, lambda h: D["XLcat"][128 * h:128 * (h + 1), :],
                        "xl")
        store_nodemajor(XLsT,
                        lambda h: D["XLcat"][N + 128 * h:N + 128 * (h + 1), :],
                        "xls")
        store_nodemajor(XRT, lambda h: D["XRtab"][128 * h:128 * (h + 1), :],
                        "xr")
        store_nodemajor(XRT, lambda h: D["T1"][128 * h:128 * (h + 1), 0:F],
                        "xrt1")
        store_nodemajor(XRsT, lambda h: D["XRStab"][128 * h:128 * (h + 1), :],
                        "xrs")
        zpad = small.tile([128, 62], F32, tag="zpad")
        nc.vector.memset(zpad[:], 0.0)
        for h in range(2):
            dma(out=D["T1"][128 * h:128 * (h + 1), 258:320], in_=zpad[:])

        # ---------------- shared GAT edge stage ----------------
        def edge_stage(xlg_tile, nslot, mask_tile, att, xr_tile, tagp):
            """xlg_tile [128, nslot*F] gathered xl rows (consumed -> w*xl).
            xr_tile [128, 1, F]; returns w tile [128, nslot, NH]."""
            xlg3 = xlg_tile[:].rearrange("p (s f) -> p s f", s=nslot)
            u = big.tile([128, nslot * F], F32, tag=tagp + "_u")
            u3 = u[:].rearrange("p (s f) -> p s f", s=nslot)
            tt(out=u3, in0=xlg3,
               in1=xr_tile[:].rearrange("p f -> p () f")
               .to_broadcast([128, nslot, F]), op=OP.add)
            lrelu(u[:], nslot * F, tagp + "_u")
            attb = att[:].rearrange("p (h f) -> p () h f", h=NH) \
                .to_broadcast([128, nslot, NH, C2])
            u4 = u[:].rearrange("p (s h f) -> p s h f", s=nslot, h=NH)
            tt(out=u4, in0=u4, in1=attb, op=OP.mult)
            lg = small.tile([128, nslot, NH], F32, tag=tagp + "_lg")
            red(out=lg[:], in_=u4, axis=AX.X, op=OP.add)
            act(out=lg[:], in_=lg[:], func=AF.Exp)
            mb = mask_tile[:].rearrange("p s -> p s ()") \
                .to_broadcast([128, nslot, NH])
            tt(out=lg[:], in0=lg[:], in1=mb, op=OP.mult)
            wb = lg[:].rearrange("p s h -> p s h ()") \
                .to_broadcast([128, nslot, NH, C2])
            xlg4 = xlg_tile[:].rearrange("p (s h f) -> p s h f", s=nslot,
                                         h=NH)
            tt(out=xlg4, in0=xlg4, in1=wb, op=OP.mult)
            return lg

        def softmax_combine(xlg_tile, lg, nslot, tagp):
            comb = small.tile([128, F + NH], F32, tag=tagp + "_comb")
            red(out=comb[:, 0:F],
                in_=xlg_tile[:].rearrange("p (s f) -> p f s", s=nslot),
                axis=AX.X, op=OP.add)
            red(out=comb[:, F:F + NH],
                in_=lg[:].rearrange("p s h -> p h s"),
                axis=AX.X, op=OP.add)
            return comb

        # ---------------- phase 1: base GAT layer 1 ----------------
        g1b_chunks = []
        for h in range(2):
            idx = small.tile([128, 4 * SLOTS * 8], I16, tag="p1_idx")
            dma(out=idx[:], in_=D["IDX_P1W"][h, :, :])
            msk = small.tile([128, 4, SLOTS], F32, tag="p1_msk")
            dma(out=msk[:], in_=D["MSK_P1"][512 * h:512 * (h + 1), :]
                .rearrange("(t p) s -> p t s", p=128))
            idxd = small.tile([128, 4, 1], I32, tag="p1_idxd")
            dma(out=idxd[:], in_=D["IDXD_P1"][512 * h:512 * (h + 1), :]
                .rearrange("(t p) s -> p t s", p=128))

            xlg = big.tile([128, 4 * SLOTS * F], F32, tag="p1_xlg")
            dgather(xlg[:].rearrange("p (k f) -> p k f", k=4 * SLOTS),
                    D["XLcat"][:], idx[:], 4 * SLOTS * 128, F)
            xrr = big.tile([128, 4, F], F32, tag="p1_xrr")
            for tpi in range(4):
                gather(out=xrr[:, tpi, :], out_offset=None, in_=D["XRtab"][:],
                       in_offset=IOA(ap=idxd[:, tpi, :], axis=0))
            dbg_dump("p1_xlg%d" % h, xlg[:])
            dbg_dump("p1_xrr%d" % h, xrr[:])

            xlg4 = xlg[:].rearrange("p (t s f) -> p t s f", t=4, s=SLOTS)
            u = big.tile([128, 4 * SLOTS * F], F32, tag="p1_u")
            u4 = u[:].rearrange("p (t s f) -> p t s f", t=4, s=SLOTS)
            tt(out=u4, in0=xlg4,
               in1=xrr[:].rearrange("p t f -> p t () f").to_broadcast([128, 4, SLOTS, F]), op=OP.add)
            lrelu(u[:], 4 * SLOTS * F, "p1_u")
            attb = att1[:].rearrange("p (h f) -> p () () h f", h=NH) \
                .to_broadcast([128, 4, SLOTS, NH, C2])
            u5 = u[:].rearrange("p (t s h f) -> p t s h f", t=4, s=SLOTS,
                                h=NH)
            tt(out=u5, in0=u5, in1=attb, op=OP.mult)
            lg = small.tile([128, 4, SLOTS, NH], F32, tag="p1_lg")
            red(out=lg[:], in_=u5, axis=AX.X, op=OP.add)
            act(out=lg[:], in_=lg[:], func=AF.Exp)
            mb = msk[:].rearrange("p t s -> p t s ()") \
                .to_broadcast([128, 4, SLOTS, NH])
            tt(out=lg[:], in0=lg[:], in1=mb, op=OP.mult)
            dbg_dump("p1_lg%d" % h, lg[:])
            wb = lg[:].rearrange("p t s h -> p t s h ()") \
                .to_broadcast([128, 4, SLOTS, NH, C2])
            xlg5 = xlg[:].rearrange("p (t s h f) -> p t s h f", t=4, s=SLOTS,
                                    h=NH)
            tt(out=xlg5, in0=xlg5, in1=wb, op=OP.mult)

            comb = small.tile([128, 4, F + NH], F32, tag="p1_comb")
            red(out=comb[:, :, 0:F],
                in_=xlg[:].rearrange("p (t s f) -> p t f s", t=4, s=SLOTS),
                axis=AX.X, op=OP.add)
            red(out=comb[:, :, F:F + NH],
                in_=lg[:].rearrange("p t s h -> p t h s"),
                axis=AX.X, op=OP.add)

            nd_ps = psum_acc.tile([128, F + NH], F32, tag="p1_ndps")
            for tp in range(4):
                nc.tensor.matmul(nd_ps[:], cmb4[:, tp, :], comb[:, tp, :],
                                 start=(tp == 0), stop=(tp == 3))
            nd = small.tile([128, F + NH], F32, tag="p1_nd")
            nc.vector.tensor_copy(out=nd[:], in_=nd_ps[:])
            dbg_dump("p1_comb%d" % h, comb[:])
            dbg_dump("p1_nd%d" % h, nd[:])
            dma(out=D["T1"][128 * h:128 * (h + 1), F:2 * F + NH], in_=nd[:])
            g1b = head_mean_bias_elu(nd[:], 128, g1bias, "p1g" + str(h))
            dbg_dump("g1b%d" % h, g1b)
            g1b_chunks.append(g1b)

        # g1_base^T -> XL2_base (T2 rows 0:256)
        g1bT = small.tile([64, 256], F32, tag="g1bT")
        for h in range(2):
            ps = psum.tile([64, 128], F32, tag="ps")
            nc.tensor.transpose(ps[:], g1b_chunks[h], ident[:])
            nc.vector.tensor_copy(out=g1bT[:, 128 * h:128 * (h + 1)],
                                  in_=ps[:])
        for h in range(2):
            ps = psum.tile([128, 128], F32, tag="ps")
            nc.tensor.matmul(ps[:], g1bT[:, 128 * h:128 * (h + 1)], w_2l[:],
                             start=True, stop=True)
            sb = small.tile([128, 128], F32, tag="p15_sb")
            nc.vector.tensor_copy(out=sb[:], in_=ps[:])
            dma(out=D["T2"][128 * h:128 * (h + 1), :], in_=sb[:])

        # ---------------- (b): full recompute of dst v ----------------
        idxb = small.tile([128, SLOTS * 8], I16, tag="b_idx")
        dma(out=idxb[:], in_=D["IDX_B_XLW"][:])
        mskb = small.tile([128, SLOTS], F32, tag="b_msk")
        dma(out=mskb[:], in_=D["MSK_B"][:])
        idxbv = small.tile([128, 1], I32, tag="b_idxv")
        dma(out=idxbv[:], in_=D["IDX_B_V"][:])
        xlgb = big.tile([128, SLOTS * F], F32, tag="b_xlg")
        dgather(xlgb[:].rearrange("p (k f) -> p k f", k=SLOTS),
                D["XLcat"][:], idxb[:], SLOTS * 128, F)
        dbg_dump("b_xlg", xlgb[:])
        xrrb = big.tile([128, F], F32, tag="b_xrr")
        gather(out=xrrb[:], out_offset=None, in_=D["XRStab"][:],
               in_offset=IOA(ap=idxbv[:], axis=0))
        dbg_dump("b_xrr", xrrb[:])
        lgb = edge_stage(xlgb, SLOTS, mskb, att1, xrrb, "b")
        dbg_dump("b_lg", lgb[:])
        combb = softmax_combine(xlgb, lgb, SLOTS, "b")
        dbg_dump("b_comb", combb[:])
        ndb_ps = psum.tile([VPC, F + NH], F32, tag="ps")
        nc.tensor.matmul(ndb_ps[:], cmb4[:, 0, 0:VPC], combb[:],
                         start=True, stop=True)
        ndb = small.tile([VPC, F + NH], F32, tag="b_nd")
        nc.vector.tensor_copy(out=ndb[:], in_=ndb_ps[:])
        g1self = head_mean_bias_elu(ndb[:], VPC, g1bias, "bg1")
        dbg_dump("g1self", g1self)

        ps_t = psum.tile([C2, VPC], F32, tag="ps")
        nc.tensor.transpose(ps_t[:], g1self, ident[:VPC, :VPC])
        g1sT = small.tile([C2, VPC], F32, tag="g1sT")
        nc.vector.tensor_copy(out=g1sT[:], in_=ps_t[:])
        ps_l = psum.tile([VPC, F], F32, tag="ps")
        nc.tensor.matmul(ps_l[:], g1sT[:], w_2l[:], start=True, stop=True)
        sb_l = small.tile([VPC, F], F32, tag="b_sbl")
        nc.vector.tensor_copy(out=sb_l[:], in_=ps_l[:])
        dma(out=D["T2"][N:N + VPC, :], in_=sb_l[:])
        ps_r = psum.tile([VPC, F], F32, tag="ps")
        nc.tensor.matmul(ps_r[:], g1sT[:], w_2r[:], start=True, stop=True)
        sb_r = small.tile([VPC, F], F32, tag="b_sbr")
        tt(out=sb_r[:], in0=ps_r[:], in1=blr[:VPC, :], op=OP.add)
        dma(out=D["XR2S"][:], in_=sb_r[:])

        # ---------------- (a): light dst updates ----------------
        idx_t1 = small.tile([128, KA * 8], I16, tag="a_it1")
        dma(out=idx_t1[:], in_=D["IDX_A_T1W"][:])
        idx_v = small.tile([128, KA * 8], I16, tag="a_iv")
        dma(out=idx_v[:], in_=D["IDX_A_VW"][:])
        idx_vs = small.tile([128, KA * 8], I16, tag="a_ivs")
        dma(out=idx_vs[:], in_=D["IDX_A_VSW"][:])
        ca = small.tile([128, KA], F32, tag="a_ca")
        dma(out=ca[:], in_=D["C_A"][:])

        t1g = big.tile([128, KA * 320], F32, tag="a_t1g")
        dgather(t1g[:].rearrange("p (k f) -> p k f", k=KA),
                D["T1"][:], idx_t1[:], KA * 128, 320)
        xlv = big.tile([128, KA * F], F32, tag="a_xlv")
        dgather(xlv[:].rearrange("p (k f) -> p k f", k=KA),
                D["XLcat"][:], idx_v[:], KA * 128, F)
        xlsv = big.tile([128, KA * F], F32, tag="a_xlsv")
        dgather(xlsv[:].rearrange("p (k f) -> p k f", k=KA),
                D["XLcat"][:], idx_vs[:], KA * 128, F)

        t1g3 = t1g[:].rearrange("p (k f) -> p k f", k=KA)  # f = 320
        t1xr = t1g3[:, :, 0:F]
        t1num = t1g3[:, :, F:2 * F]
        t1den = t1g3[:, :, 2 * F:2 * F + NH]
        xlv3 = xlv[:].rearrange("p (k f) -> p k f", k=KA)
        xlsv3 = xlsv[:].rearrange("p (k f) -> p k f", k=KA)

        def logits_expC(xl3, tg):
            u = big.tile([128, KA * F], F32, tag="a_u" + tg)
            u3 = u[:].rearrange("p (k f) -> p k f", k=KA)
            tt(out=u3, in0=xl3, in1=t1xr, op=OP.add)
            lrelu(u[:], KA * F, "a_u" + tg)
            attb = att1[:].rearrange("p (h f) -> p () h f", h=NH) \
                .to_broadcast([128, KA, NH, C2])
            u4 = u[:].rearrange("p (k h f) -> p k h f", k=KA, h=NH)
            tt(out=u4, in0=u4, in1=attb, op=OP.mult)
            lw = small.tile([128, KA, NH], F32, tag="a_lw" + tg)
            red(out=lw[:], in_=u4, axis=AX.X, op=OP.add)
            act(out=lw[:], in_=lw[:], func=AF.Exp)
            cb = ca[:].rearrange("p k -> p k ()").to_broadcast([128, KA, NH])
            tt(out=lw[:], in0=lw[:], in1=cb, op=OP.mult)
            return lw

        dbg_dump("a_t1g", t1g[:])
        dbg_dump("a_xlv", xlv[:])
        dbg_dump("a_xlsv", xlsv[:])
        wn = logits_expC(xlsv3, "n")    # C * w_new
        wo = logits_expC(xlv3, "o")     # C * w_old
        dbg_dump("a_wn", wn[:])
        dbg_dump("a_wo", wo[:])

        dden = small.tile([128, KA, NH], F32, tag="a_dden")
        tt(out=dden[:], in0=wn[:], in1=wo[:], op=OP.subtract)
        tt(out=dden[:], in0=dden[:], in1=t1den, op=OP.add)
        wnb = wn[:].rearrange("p k h -> p k h ()") \
            .to_broadcast([128, KA, NH, C2])
        xlsv4 = xlsv[:].rearrange("p (k h f) -> p k h f", k=KA, h=NH)
        tt(out=xlsv4, in0=xlsv4, in1=wnb, op=OP.mult)
        wob = wo[:].rearrange("p k h -> p k h ()") \
            .to_broadcast([128, KA, NH, C2])
        xlv4 = xlv[:].rearrange("p (k h f) -> p k h f", k=KA, h=NH)
        tt(out=xlv4, in0=xlv4, in1=wob, op=OP.mult)
        tt(out=xlsv3, in0=xlsv3, in1=xlv3, op=OP.subtract)
        tt(out=xlsv3, in0=xlsv3, in1=t1num, op=OP.add)
        nc.vector.reciprocal(out=dden[:], in_=dden[:])
        ddb = dden[:].rearrange("p k h -> p k h ()") \
            .to_broadcast([128, KA, NH, C2])
        tt(out=xlsv4, in0=xlsv4, in1=ddb, op=OP.mult)
        radd = big.tile([128, KA, C2], F32, tag="a_radd")
        tt(out=radd[:], in0=xlsv4[:, :, 0, :], in1=xlsv4[:, :, 1, :],
           op=OP.add)
        ts_mul(radd[:], radd[:], 0.5)
        g1bb = g1bias[:].rearrange("p f -> p () f").to_broadcast(
            [128, KA, C2])
        tt(out=radd[:], in0=radd[:], in1=g1bb, op=OP.add)
        radd_flat = radd[:].rearrange("p k f -> p (k f)")
        elu_inplace(radd_flat, big, KA * C2, "a_elu")
        dma(out=D["G1L"][:].rearrange("(p k) f -> p k f", p=128), in_=radd[:])

        # ---------------- rare light rows -> T2 rows 288: ----------------
        idx_r = small.tile([128, 1], I32, tag="r_idx")
        dma(out=idx_r[:], in_=D["IDX_RARE"][:])
        grare = small.tile([128, C2], F32, tag="r_g")
        gather(out=grare[:], out_offset=None, in_=D["G1L"][:],
               in_offset=IOA(ap=idx_r[:], axis=0))
        ps_rt = psum.tile([C2, 128], F32, tag="ps")
        nc.tensor.transpose(ps_rt[:], grare[:], ident[:])
        grT = small.tile([C2, 128], F32, tag="grT")
        nc.vector.tensor_copy(out=grT[:], in_=ps_rt[:])
        ps_rm = psum.tile([128, F], F32, tag="ps")
        nc.tensor.matmul(ps_rm[:], grT[:], w_2l[:], start=True, stop=True)
        sb_rm = small.tile([128, F], F32, tag="r_sb")
        nc.vector.tensor_copy(out=sb_rm[:], in_=ps_rm[:])
        dma(out=D["T2"][N + VPC:N + VPC + 128, :], in_=sb_rm[:])

        # ---------------- (d): layer 2 at dst v ----------------
        idxd2 = small.tile([128, SLOTS * 8], I16, tag="d_idx")
        dma(out=idxd2[:], in_=D["IDX_D_T2W"][:])
        mskd = small.tile([128, SLOTS], F32, tag="d_msk")
        dma(out=mskd[:], in_=D["MSK_D"][:])
        idxdv = small.tile([128, 1], I32, tag="d_idxv")
        dma(out=idxdv[:], in_=D["IDX_D_V"][:])
        xl2g = big.tile([128, SLOTS * F], F32, tag="d_xlg")
        dgather(xl2g[:].rearrange("p (k f) -> p k f", k=SLOTS),
                D["T2"][:], idxd2[:], SLOTS * 128, F)
        xr2r = big.tile([128, F], F32, tag="d_xrr")
        gather(out=xr2r[:], out_offset=None, in_=D["XR2S"][:],
               in_offset=IOA(ap=idxdv[:], axis=0))
        dbg_dump("d_xlg", xl2g[:])
        dbg_dump("d_xrr", xr2r[:])
        lgd = edge_stage(xl2g, SLOTS, mskd, att2, xr2r, "d")
        dbg_dump("d_lg", lgd[:])
        combd = softmax_combine(xl2g, lgd, SLOTS, "d")
        ndd_ps = psum.tile([VPC, F + NH], F32, tag="ps")
        nc.tensor.matmul(ndd_ps[:], cmb4[:, 0, 0:VPC], combd[:],
                         start=True, stop=True)
        ndd = small.tile([VPC, F + NH], F32, tag="d_nd")
        nc.vector.tensor_copy(out=ndd[:], in_=ndd_ps[:])
        g2row = head_mean_bias_elu(ndd[:], VPC, g2bias, "dg2")
        dbg_dump("g2row", g2row)

        # out = tanh(g2row @ rec_w + rec_b)
        ps_ot = psum.tile([C2, VPC], F32, tag="ps")
        nc.tensor.transpose(ps_ot[:], g2row, ident[:VPC, :VPC])
        g2T = small.tile([C2, VPC], F32, tag="g2T")
        nc.vector.tensor_copy(out=g2T[:], in_=ps_ot[:])
        ps_om = psum.tile([C2, VPC], F32, tag="ps")
        nc.tensor.matmul(ps_om[:], w_rec[:], g2T[:], start=True, stop=True)
        outT = small.tile([C2, VPC], F32, tag="outT")
        act(out=outT[:], in_=ps_om[:], func=AF.Tanh, bias=b_rec[:])
        ps_of = psum.tile([VPC, C2], F32, tag="ps")
        nc.tensor.transpose(ps_of[:], outT[:], ident[:C2, :C2])
        outsb = small.tile([VPC, C2], F32, tag="outsb")
        nc.vector.tensor_copy(out=outsb[:], in_=ps_of[:])
        dma(out=D["out"][:], in_=outsb[:])
        if dbg:
            dbg_dump("XLcat", D["XLcat"][:])
            dbg_dump("T1", D["T1"][:])
            dbg_dump("T2", D["T2"][:])
            dbg_dump("XR2S", D["XR2S"][:])
            dbg_dump("G1L", D["G1L"][:])


# --------------------------------------------------------------------------
# Entry point
# --------------------------------------------------------------------------

def _make_in_maps(inputs, shared, percore):
    f32 = np.float32

    def rep(v, shape):
        return np.ascontiguousarray(
            np.broadcast_to(np.asarray(v, f32).reshape(shape),
                            (128,) + tuple(shape[1:])))

    base = {
        "x": np.ascontiguousarray(inputs["x"], f32),
        "E_emb": np.ascontiguousarray(inputs["E_emb"], f32),
        "conv_b": np.ascontiguousarray(inputs["conv_b"].reshape(128, 1), f32),
        "lin2_b": np.ascontiguousarray(inputs["lin2_b"].reshape(64, 1), f32),
        "g1_bl": np.ascontiguousarray(inputs["g1_bl"].reshape(128, 1), f32),
        "g1_br": np.ascontiguousarray(inputs["g1_br"].reshape(128, 1), f32),
        "rec_b": np.ascontiguousarray(inputs["rec_b"].reshape(64, 1), f32),
        "att1_rep": rep(inputs["g1_att"], (1, F)),
        "att2_rep": rep(inputs["g2_att"], (1, F)),
        "g1bias_rep": rep(inputs["g1_bias"], (1, C2)),
        "g2bias_rep": rep(inputs["g2_bias"], (1, C2)),
        "blr_rep": rep(inputs["g2_bl"] + inputs["g2_br"], (1, F)),
    }
    for nm in ("node_proj", "emb_proj", "conv_w0", "conv_w1", "lin2_w",
               "masked_proj", "normal_proj", "g1_wl", "g1_wr", "g2_wl",
               "g2_wr", "rec_w"):
        base[nm] = np.ascontiguousarray(inputs[nm], f32)
    base.update({k: np.ascontiguousarray(v) for k, v in shared.items()})
    in_maps = []
    for c in range(NCORES):
        m = dict(base)
        m.update({k: np.ascontiguousarray(v) for k, v in percore[c].items()})
        in_maps.append(m)
    return in_maps


_CACHE = {}
TRACE = False          # set by test.py to capture NTFF profiles
LRELU_ACT = False      # ACT Lrelu mis-handles alpha on HW; use DVE mul+max
LAST_RESULT = None


def kernel(**inputs):
    global LAST_RESULT
    inputs = {k: np.asarray(v) for k, v in inputs.items()}
    shared, percore, dims = _build_tables(inputs["edge_index"])
    key = (dims["SLOTS"], dims["K2"], LRELU_ACT)
    if key not in _CACHE:
        _CACHE[key] = _build_program(dims, lrelu_act=LRELU_ACT)
    nc = _CACHE[key]
    in_maps = _make_in_maps(inputs, shared, percore)
    kw = {}
    if TRACE:
        kw = dict(trace=True, trace_cores=list(range(NCORES)))
    res = run_bass_kernel_spmd(nc, in_maps, core_ids=list(range(NCORES)),
                               **kw)
    LAST_RESULT = res
    out = np.concatenate([res.results[c]["out"] for c in range(NCORES)],
                         axis=0)
    return out.astype(np.float32)



# revision 3
# speedup vs baseline: 1.5917x; 1.5917x over previous
"""Trainium2 Bass kernel for nn_NodeDetector (masked-node GATv2 ensemble).

Algorithm: the reference vmaps a full 2-layer GATv2 over 256 "masked node"
variants, but variant v differs from the shared base computation in exactly
one input row (row v).  We compute the base graph once and apply sparse
incremental updates per variant:

  phase 0  dense projections -> XL/XR (base rows) and XLs/XRs (masked rows)
  phase 1  base GAT layer 1: per-dst softmax sums (num1/den1) + g1_base
  (a)      per variant v: "light" g1 updates at out-neighbors d of v
           (only edges v->d changed: closed-form num/den delta)
  (b)      per variant v: full recompute of g1 at node v
  (d)      layer 2 at dst v only: gather xl2 of in-neighbors (base / self /
           rare light rows), one softmax, project + tanh.

Attention softmaxes skip the per-dst max subtraction (mathematically
identical; logits are O(10) so fp32 exp is safe).  All gathers use
host-built index tables (edge_index is host data) via gpsimd indirect DMA.
Work is sharded 32 variants per core across 8 cores; phases 0/1 are
replicated per core.  No collectives.

Perf notes vs the first working version:
  - edge slots are degree-aware bin-packed (variable rows per dst) instead
    of a fixed ceil(max_deg/4) per row: ~40% less gather + DVE volume.
  - all constants/index tables ship in 4 packed DMAs (dma_start issue on
    the sync queue costs ~600ns each; the old code had ~35 of them).
  - per-half p1 tiles get distinct tags so half 1's gather is not
    WAR-serialized against half 0's vector reads.
  - node-major XL/XR tables are stored with one strided DMA each.
"""

import numpy as np

import concourse.bass as bass
import concourse.mybir as mybir
import concourse.tile as tile
from concourse import bacc
from concourse.bass_utils import run_bass_kernel_spmd
from concourse.masks import make_identity

F32 = mybir.dt.float32
I32 = mybir.dt.int32
I16 = mybir.dt.int16
AF = mybir.ActivationFunctionType
OP = mybir.AluOpType
AX = mybir.AxisListType

N = 256          # nodes / variants
F = 128          # NUM_HEAD * C2
C2 = 64
NH = 2
NCORES = 8
VPC = N // NCORES   # variants per core = 32
NEG = 0.2           # leaky relu slope


# --------------------------------------------------------------------------
# Host-side table construction
# --------------------------------------------------------------------------

def _wrap16(flat):
    """int16 idx layout for dma_gather: value for flat position i lives
    at [i % 16, i // 16], tiled to 128 partitions."""
    flat = np.asarray(flat)
    num = flat.shape[0]
    A = np.zeros((16, num // 16), np.int16)
    A[np.arange(num) % 16, np.arange(num) // 16] = flat.astype(np.int16)
    return np.ascontiguousarray(np.tile(A, (8, 1)))


def _wrapPK(idx_pk):
    """[128, K] logical idx (out[p, k] = tab[idx_pk[p,k]]) -> wrapped."""
    return _wrap16(idx_pk.T.reshape(-1))


def _min_slots(degs, nrows):
    S = 1
    while sum(-(-d // S) for d in degs) > nrows:
        S += 1
    return S


def _binpack(dst_list, in_edges, nrows, S):
    """Rows of (dst, edge_id_chunk), <= S edges each, padded to nrows."""
    rows = []
    for d in dst_list:
        el = in_edges[d]
        for i in range(0, len(el), S):
            rows.append((d, el[i:i + S]))
    assert len(rows) <= nrows
    rows += [None] * (nrows - len(rows))
    return rows


def _build_tables(edge_index):
    src = edge_index[0].astype(np.int64)
    dst = edge_index[1].astype(np.int64)
    E = src.shape[0]

    in_edges = [[] for _ in range(N)]
    for e in range(E):
        in_edges[dst[e]].append(e)

    # p1: two halves of 128 dst, 512 rows each, degree-aware slot packing
    S1 = max(_min_slots([len(in_edges[d]) for d in range(128 * h,
                                                         128 * (h + 1))], 512)
             for h in range(2))
    # b/d: per-core 32 dst over 128 rows; one global slot count
    SB = max(_min_slots([len(in_edges[v]) for v in range(VPC * c,
                                                         VPC * (c + 1))], 128)
             for c in range(NCORES))

    out_by_src = [[] for _ in range(N)]
    for e in range(E):
        if dst[e] != src[e]:
            out_by_src[src[e]].append(int(dst[e]))
    light = []
    for v in range(N):
        cnt = {}
        for d in out_by_src[v]:
            cnt[d] = cnt.get(d, 0) + 1
        light.append(sorted(cnt.items()))
    max_light = max(len(l) for l in light)
    K2 = 4 * (-(-max_light // 4))       # light slots per variant, mult of 4
    KA = K2 * VPC // 128                # light slots per partition

    shared = {}
    IDX1W = np.zeros((2, 128, 4 * S1 * 8), np.int16)
    MSK1 = np.zeros((2, 128, 4 * S1), np.float32)
    IDXD1 = np.zeros((2, 128, 4), np.int32)
    CMB1 = np.zeros((2, 128, 4, 128), np.float32)
    for h in range(2):
        rows = _binpack(range(128 * h, 128 * (h + 1)), in_edges, 512, S1)
        pk = np.zeros((128, 4 * S1), np.int64)
        for r, ent in enumerate(rows):
            t, p = divmod(r, 128)
            if ent is None:
                continue
            d, el = ent
            IDXD1[h, p, t] = d
            CMB1[h, p, t, d - 128 * h] = 1.0
            for si, e in enumerate(el):
                pk[p, t * S1 + si] = src[e]
                MSK1[h, p, t * S1 + si] = 1.0
        IDX1W[h] = _wrapPK(pk)
    shared["IDX1W"] = IDX1W
    shared["MSK1"] = MSK1
    shared["IDXD1"] = IDXD1
    shared["CMB1"] = CMB1

    percore = []
    for c in range(NCORES):
        t = {}
        V = list(range(c * VPC, (c + 1) * VPC))

        rows = _binpack(V, in_edges, 128, SB)
        IDXB = np.zeros((128, SB), np.int64)
        IDXD2 = np.zeros((128, SB), np.int64)
        MSKBD = np.zeros((128, SB), np.float32)
        CMBBD = np.zeros((128, VPC), np.float32)
        IDX_B_V = np.zeros((128, 1), np.int32)
        IDX_D_V = np.zeros((128, 1), np.int32)
        IDX_RARE = np.zeros((128, 1), np.int32)
        rare_map = {}
        for r, ent in enumerate(rows):
            if ent is None:
                continue
            v, el = ent
            vi = v - c * VPC
            CMBBD[r, vi] = 1.0
            IDX_B_V[r, 0] = 256 + v     # XRcat self half
            IDX_D_V[r, 0] = vi          # XR2S row
            lpos = {d: i for i, (d, _) in enumerate(light[v])}
            for si, e in enumerate(el):
                sn = int(src[e])
                MSKBD[r, si] = 1.0
                IDXB[r, si] = 256 + v if sn == v else sn
                if sn == v:
                    IDXD2[r, si] = 256 + vi
                elif sn in lpos:
                    key = (vi, sn)
                    if key not in rare_map:
                        rs = len(rare_map)
                        assert rs < 128, "rare-row overflow"
                        rare_map[key] = rs
                        IDX_RARE[rs, 0] = vi * K2 + lpos[sn]
                    IDXD2[r, si] = 288 + rare_map[key]
                else:
                    IDXD2[r, si] = sn
        t["IDX_BW"] = _wrapPK(IDXB)
        t["IDX_DW"] = _wrapPK(IDXD2)
        t["MSKBD"] = MSKBD
        t["CMBBD"] = CMBBD
        t["IDX_B_V"] = IDX_B_V
        t["IDX_D_V"] = IDX_D_V
        t["IDX_RARE"] = IDX_RARE

        IDX_A_T1 = np.zeros((128, KA), np.int64)
        IDX_A_V = np.zeros((128, KA), np.int64)
        IDX_A_VS = np.zeros((128, KA), np.int64)
        C_A = np.zeros((128, KA), np.float32)
        for r in range(128 * KA):
            vi, slot = divmod(r, K2)
            p, k = divmod(r, KA)
            v = V[vi]
            IDX_A_V[p, k] = v
            IDX_A_VS[p, k] = 256 + v
            if slot < len(light[v]):
                d, cc = light[v][slot]
                IDX_A_T1[p, k] = d
                C_A[p, k] = float(cc)
        t["IDX_A_T1W"] = _wrapPK(IDX_A_T1)
        t["IDX_A_VW"] = _wrapPK(IDX_A_V)
        t["IDX_A_VSW"] = _wrapPK(IDX_A_VS)
        t["C_A"] = C_A
        percore.append(t)

    dims = dict(S1=S1, SB=SB, K2=K2, KA=KA)
    return shared, percore, dims


# --------------------------------------------------------------------------
# Packed-input layouts (single source of truth for device + host)
# --------------------------------------------------------------------------

def _pack_layout(dims):
    S1, SB, KA = dims["S1"], dims["SB"], dims["KA"]
    pf = [("node_proj", 64, 128), ("emb_proj", 64, 128),
          ("conv_w0", 128, 128), ("conv_w1", 128, 128), ("conv_b", 128, 1),
          ("lin2_w", 128, 64), ("lin2_b", 64, 1), ("masked_proj", 64, 64),
          ("normal_proj", 64, 64), ("g1_wl", 64, 128), ("g1_bl", 128, 1),
          ("g1_wr", 64, 128), ("g1_br", 128, 1), ("g2_wl", 64, 128),
          ("g2_wr", 64, 128), ("rec_w", 64, 64), ("rec_b", 64, 1),
          ("att1", 128, 128), ("att2", 128, 128), ("g1bias", 128, 64),
          ("g2bias", 128, 64), ("blr", 128, 128),
          ("CMB1", 128, 8 * 128), ("CMBBD", 128, VPC),
          ("MSK1", 128, 8 * S1), ("MSKBD", 128, SB), ("C_A", 128, KA)]
    pi16 = [("IDX1W0", 128, 4 * S1 * 8), ("IDX1W1", 128, 4 * S1 * 8),
            ("IDX_BW", 128, SB * 8), ("IDX_DW", 128, SB * 8),
            ("IDX_A_T1W", 128, KA * 8), ("IDX_A_VW", 128, KA * 8),
            ("IDX_A_VSW", 128, KA * 8)]
    pi32 = [("IDXD1", 128, 8), ("IDX_B_V", 128, 1), ("IDX_D_V", 128, 1),
            ("IDX_RARE", 128, 1)]

    def offsets(lst):
        off, o = {}, 0
        for nm, rows, cols in lst:
            off[nm] = (o, rows, cols)
            o += cols
        return off, o

    return offsets(pf), offsets(pi16), offsets(pi32)


# --------------------------------------------------------------------------
# Device program
# --------------------------------------------------------------------------

def _build_program(dims):
    S1, SB, K2, KA = dims["S1"], dims["SB"], dims["K2"], dims["KA"]
    (off_f, nf), (off_i16, ni16), (off_i32, ni32) = _pack_layout(dims)

    nc = bacc.Bacc("TRN2", target_bir_lowering=False, debug=False)

    D = {}
    D["packf"] = nc.dram_tensor("packf", [128, nf], F32,
                                kind="ExternalInput")
    D["packi16"] = nc.dram_tensor("packi16", [128, ni16], I16,
                                  kind="ExternalInput")
    D["packi32"] = nc.dram_tensor("packi32", [128, ni32], I32,
                                  kind="ExternalInput")
    D["xE"] = nc.dram_tensor("xE", [128, 256], F32, kind="ExternalInput")

    D["out"] = nc.dram_tensor("out", [VPC, 64], F32, kind="ExternalOutput")
    D["XLcat"] = nc.dram_tensor("XLcat", [2 * N, F], F32)
    D["XRcat"] = nc.dram_tensor("XRcat", [2 * N, F], F32)
    D["T1"] = nc.dram_tensor("T1", [N, 320], F32)
    D["G1L"] = nc.dram_tensor("G1L", [VPC * K2, C2], F32)
    D["T2"] = nc.dram_tensor("T2", [N + VPC + 128, F], F32)
    D["XR2S"] = nc.dram_tensor("XR2S", [VPC, F], F32)

    with tile.TileContext(nc) as tc:
        _trace(nc, tc, D, dims, (off_f, off_i16, off_i32))
    nc.compile()
    return nc


def _trace(nc, tc, D, dims, offs):
    S1, SB, K2, KA = dims["S1"], dims["SB"], dims["K2"], dims["KA"]
    off_f, off_i16, off_i32 = offs
    import contextlib
    ctx = contextlib.ExitStack()
    with ctx:
        consts = ctx.enter_context(tc.tile_pool(name="consts", bufs=1))
        small = ctx.enter_context(tc.tile_pool(name="small", bufs=1))
        big = ctx.enter_context(tc.tile_pool(name="big", bufs=1))
        psum = ctx.enter_context(tc.tile_pool(name="psum", bufs=4,
                                              space="PSUM"))
        psum_acc = ctx.enter_context(tc.tile_pool(name="psacc", bufs=2,
                                                  space="PSUM"))

        dma = nc.sync.dma_start

        def dgather(out_ap, in_ap, idx_ap, num, elem):
            nc.gpsimd.dma_gather(out_ap=out_ap, in_ap=in_ap, idxs_ap=idx_ap,
                                 num_idxs=num, num_idxs_reg=num,
                                 elem_size=elem, single_packet=False)
        tt = nc.vector.tensor_tensor
        red = nc.vector.tensor_reduce
        act = nc.scalar.activation
        gather = nc.gpsimd.indirect_dma_start
        IOA = bass.IndirectOffsetOnAxis

        # ---------------- constants (4 packed DMAs) ----------------
        ident = consts.tile([128, 128], F32, tag="ident")
        make_identity(nc, ident[:])

        nfc = sum(c for _, (_, _, c) in
                  [(k, v) for k, v in off_f.items()]) if False else None
        packf = consts.tile([128, D["packf"].shape[1]], F32, tag="packf")
        dma(out=packf[:], in_=D["packf"][:])
        packi16 = consts.tile([128, D["packi16"].shape[1]], I16,
                              tag="packi16")
        dma(out=packi16[:], in_=D["packi16"][:])
        packi32 = consts.tile([128, D["packi32"].shape[1]], I32,
                              tag="packi32")
        dma(out=packi32[:], in_=D["packi32"][:])
        xE = consts.tile([128, 256], F32, tag="xE")
        dma(out=xE[:], in_=D["xE"][:])

        def cv(name):
            o, rows, cols = off_f[name]
            return packf[:rows, o:o + cols]

        def iv16(name):
            o, rows, cols = off_i16[name]
            return packi16[:rows, o:o + cols]

        def iv32(name):
            o, rows, cols = off_i32[name]
            return packi32[:rows, o:o + cols]

        # ---------------- helpers ----------------
        def ts_mul(out, in0, s):
            nc.vector.tensor_scalar_mul(out=out, in0=in0, scalar1=s)

        def lrelu(flat_ap, nfree, tag):
            t_ = big.tile([128, nfree], F32, tag="lr_" + tag)
            ta = t_[:flat_ap.shape[0], :]
            ts_mul(ta, flat_ap, NEG)
            tt(out=flat_ap, in0=flat_ap, in1=ta, op=OP.max)

        def elu_inplace(x_ap, scratch_pool, nfree, tag):
            xpos = scratch_pool.tile([128, nfree], F32, tag=tag + "_xp")
            nrow = x_ap.shape[0]
            xp = xpos[:nrow, :]
            nc.vector.tensor_scalar_max(out=xp, in0=x_ap, scalar1=0.0)
            nc.vector.tensor_scalar_min(out=x_ap, in0=x_ap, scalar1=0.0)
            act(out=x_ap, in_=x_ap, func=AF.Exp)
            nc.vector.tensor_scalar_add(out=x_ap, in0=x_ap, scalar1=-1.0)
            nc.vector.tensor_add(out=x_ap, in0=x_ap, in1=xp)
            return x_ap

        def head_mean_bias_elu(nd_ap, nrow, bias_rep, tag):
            """nd_ap [nrow, F+NH] = (num|den) -> elu(mean_h(num/den)+bias)."""
            rec = small.tile([128, NH], F32, tag=tag + "_rec")
            nc.vector.reciprocal(out=rec[:nrow, :], in_=nd_ap[:, F:F + NH])
            r0 = small.tile([128, C2], F32, tag=tag + "_r0")
            r1 = small.tile([128, C2], F32, tag=tag + "_r1")
            ts_mul(r0[:nrow, :], nd_ap[:, 0:C2], rec[:nrow, 0:1])
            ts_mul(r1[:nrow, :], nd_ap[:, C2:F], rec[:nrow, 1:2])
            tt(out=r0[:nrow, :], in0=r0[:nrow, :], in1=r1[:nrow, :], op=OP.add)
            ts_mul(r0[:nrow, :], r0[:nrow, :], 0.5)
            tt(out=r0[:nrow, :], in0=r0[:nrow, :], in1=bias_rep[:nrow, :],
               op=OP.add)
            return elu_inplace(r0[:nrow, :], small, C2, tag)

        # ---------------- phase 0 ----------------
        def mm_to_sbuf(lhsT, rhs, M, Nf, tag, bias=None, func=AF.Identity,
                       extra=None):
            out_tile = small.tile([M, Nf], F32, tag=tag)
            ps = psum.tile([128, 256], F32, tag="ps")
            nc.tensor.matmul(ps[:M, :Nf], lhsT, rhs, start=True,
                             stop=extra is None)
            if extra is not None:
                nc.tensor.matmul(ps[:M, :Nf], extra[0], extra[1],
                                 start=False, stop=True)
            if bias is None:
                act(out=out_tile[:], in_=ps[:M, :Nf], func=func)
            else:
                act(out=out_tile[:], in_=ps[:M, :Nf], func=func, bias=bias)
            return out_tile

        xT = small.tile([64, 256], F32, tag="xT")
        eT = small.tile([64, 256], F32, tag="eT")
        for h in range(2):
            for (col0, dstT) in ((64 * h, xT), (128 + 64 * h, eT)):
                pst = psum.tile([64, 128], F32, tag="ps")
                nc.tensor.transpose(pst[:], xE[:, col0:col0 + 64], ident[:])
                nc.vector.tensor_copy(out=dstT[:, 128 * h:128 * (h + 1)],
                                      in_=pst[:])

        xpT = mm_to_sbuf(cv("node_proj"), xT[:], 128, 256, "xpT")
        epT = mm_to_sbuf(cv("emb_proj"), eT[:], 128, 256, "epT")
        HbT = mm_to_sbuf(cv("conv_w0"), epT[:], 128, 256, "HbT",
                         bias=cv("conv_b"), func=AF.Tanh,
                         extra=(cv("conv_w1"), xpT[:]))
        HsT = mm_to_sbuf(cv("conv_w0"), epT[:], 128, 256, "HsT",
                         bias=cv("conv_b"), func=AF.Tanh)
        MbT = mm_to_sbuf(cv("lin2_w"), HbT[:], 64, 256, "MbT",
                         bias=cv("lin2_b"))
        MsT = mm_to_sbuf(cv("lin2_w"), HsT[:], 64, 256, "MsT",
                         bias=cv("lin2_b"))
        PbT = mm_to_sbuf(cv("normal_proj"), MbT[:], 64, 256, "PbT")
        PsT = mm_to_sbuf(cv("masked_proj"), MsT[:], 64, 256, "PsT")
        XLT = mm_to_sbuf(cv("g1_wl"), PbT[:], 128, 256, "XLT",
                         bias=cv("g1_bl"))
        XRT = mm_to_sbuf(cv("g1_wr"), PbT[:], 128, 256, "XRT",
                         bias=cv("g1_br"))
        XLsT = mm_to_sbuf(cv("g1_wl"), PsT[:], 128, 256, "XLsT",
                          bias=cv("g1_bl"))
        XRsT = mm_to_sbuf(cv("g1_wr"), PsT[:], 128, 256, "XRsT",
                          bias=cv("g1_br"))

        # node-major tables, one strided DMA per table
        def cat_store(srcTs, dram):
            sb = small.tile([128, 4 * 128], F32, tag="cat_" + dram.name)
            for k, (srcT, h) in enumerate(srcTs):
                ps = psum.tile([128, 128], F32, tag="ps")
                nc.tensor.transpose(ps[:], srcT[:, 128 * h:128 * (h + 1)],
                                    ident[:])
                nc.vector.tensor_copy(out=sb[:, 128 * k:128 * (k + 1)],
                                      in_=ps[:])
            dma(out=dram[:].rearrange("(c p) f -> p c f", p=128),
                in_=sb[:].rearrange("p (c f) -> p c f", c=4))
            return sb

        cat_store([(XLT, 0), (XLT, 1), (XLsT, 0), (XLsT, 1)], D["XLcat"])
        xrcat_sb = cat_store([(XRT, 0), (XRT, 1), (XRsT, 0), (XRsT, 1)],
                             D["XRcat"])
        for h in range(2):
            dma(out=D["T1"][128 * h:128 * (h + 1), 0:F],
                in_=xrcat_sb[:, 128 * h:128 * (h + 1)])

        # ---------------- early gathers (gpsimd queue order) -------------
        xlg1 = []
        xrr1 = []
        for h in range(2):
            xlg = big.tile([128, 4 * S1 * F], F32, tag="p1_xlg%d" % h)
            dgather(xlg[:].rearrange("p (k f) -> p k f", k=4 * S1),
                    D["XLcat"][:], iv16("IDX1W%d" % h), 4 * S1 * 128, F)
            xrr = big.tile([128, 4, F], F32, tag="p1_xrr%d" % h)
            for t in range(4):
                gather(out=xrr[:, t, :], out_offset=None, in_=D["XRcat"][:],
                       in_offset=IOA(ap=iv32("IDXD1")[:, 4 * h + t:
                                                      4 * h + t + 1],
                                     axis=0))
            xlg1.append(xlg)
            xrr1.append(xrr)

        xlgb = big.tile([128, SB * F], F32, tag="b_xlg")
        dgather(xlgb[:].rearrange("p (k f) -> p k f", k=SB),
                D["XLcat"][:], iv16("IDX_BW"), SB * 128, F)
        xrrb = big.tile([128, F], F32, tag="b_xrr")
        gather(out=xrrb[:], out_offset=None, in_=D["XRcat"][:],
               in_offset=IOA(ap=iv32("IDX_B_V"), axis=0))

        xlv = big.tile([128, KA * F], F32, tag="a_xlv")
        dgather(xlv[:].rearrange("p (k f) -> p k f", k=KA),
                D["XLcat"][:], iv16("IDX_A_VW"), KA * 128, F)
        xlsv = big.tile([128, KA * F], F32, tag="a_xlsv")
        dgather(xlsv[:].rearrange("p (k f) -> p k f", k=KA),
                D["XLcat"][:], iv16("IDX_A_VSW"), KA * 128, F)

        # ---------------- shared GAT edge stage ----------------
        def edge_stage(xlg_tile, nslot, mask_ap, att, xr_tile, tagp):
            """xlg_tile [128, nslot*F] gathered xl rows (consumed -> w*xl).
            xr_tile [128, F]; returns w tile [128, nslot, NH]."""
            xlg3 = xlg_tile[:].rearrange("p (s f) -> p s f", s=nslot)
            u = big.tile([128, nslot * F], F32, tag=tagp + "_u")
            u3 = u[:].rearrange("p (s f) -> p s f", s=nslot)
            tt(out=u3, in0=xlg3,
               in1=xr_tile[:].rearrange("p f -> p () f")
               .to_broadcast([128, nslot, F]), op=OP.add)
            lrelu(u[:], nslot * F, tagp + "_u")
            attb = att.rearrange("p (h f) -> p () h f", h=NH) \
                .to_broadcast([128, nslot, NH, C2])
            u4 = u[:].rearrange("p (s h f) -> p s h f", s=nslot, h=NH)
            tt(out=u4, in0=u4, in1=attb, op=OP.mult)
            lg = small.tile([128, nslot, NH], F32, tag=tagp + "_lg")
            red(out=lg[:], in_=u4, axis=AX.X, op=OP.add)
            act(out=lg[:], in_=lg[:], func=AF.Exp)
            mb = mask_ap.rearrange("p s -> p s ()") \
                .to_broadcast([128, nslot, NH])
            tt(out=lg[:], in0=lg[:], in1=mb, op=OP.mult)
            wb = lg[:].rearrange("p s h -> p s h ()") \
                .to_broadcast([128, nslot, NH, C2])
            xlg4 = xlg_tile[:].rearrange("p (s h f) -> p s h f", s=nslot,
                                         h=NH)
            tt(out=xlg4, in0=xlg4, in1=wb, op=OP.mult)
            return lg

        def softmax_combine(xlg_tile, lg, nslot, tagp):
            comb = small.tile([128, F + NH], F32, tag=tagp + "_comb")
            red(out=comb[:, 0:F],
                in_=xlg_tile[:].rearrange("p (s f) -> p f s", s=nslot),
                axis=AX.X, op=OP.add)
            red(out=comb[:, F:F + NH],
                in_=lg[:].rearrange("p s h -> p h s"),
                axis=AX.X, op=OP.add)
            return comb

        # ---------------- phase 1: base GAT layer 1 ----------------
        att1 = cv("att1")
        g1b_chunks = []
        for h in range(2):
            xlg = xlg1[h]
            xrr = xrr1[h]
            tg = "p1h%d" % h
            xlg4 = xlg[:].rearrange("p (t s f) -> p t s f", t=4, s=S1)
            u = big.tile([128, 4 * S1 * F], F32, tag=tg + "_u")
            u4 = u[:].rearrange("p (t s f) -> p t s f", t=4, s=S1)
            tt(out=u4, in0=xlg4,
               in1=xrr[:].rearrange("p t f -> p t () f")
               .to_broadcast([128, 4, S1, F]), op=OP.add)
            lrelu(u[:], 4 * S1 * F, tg + "_u")
            attb = att1.rearrange("p (h f) -> p () () h f", h=NH) \
                .to_broadcast([128, 4, S1, NH, C2])
            u5 = u[:].rearrange("p (t s h f) -> p t s h f", t=4, s=S1,
                                h=NH)
            tt(out=u5, in0=u5, in1=attb, op=OP.mult)
            lg = small.tile([128, 4, S1, NH], F32, tag=tg + "_lg")
            red(out=lg[:], in_=u5, axis=AX.X, op=OP.add)
            act(out=lg[:], in_=lg[:], func=AF.Exp)
            msk = cv("MSK1")[:, 4 * S1 * h:4 * S1 * (h + 1)] \
                .rearrange("p (t s) -> p t s", t=4)
            mb = msk.rearrange("p t s -> p t s ()") \
                .to_broadcast([128, 4, S1, NH])
            tt(out=lg[:], in0=lg[:], in1=mb, op=OP.mult)
            wb = lg[:].rearrange("p t s h -> p t s h ()") \
                .to_broadcast([128, 4, S1, NH, C2])
            xlg5 = xlg[:].rearrange("p (t s h f) -> p t s h f", t=4, s=S1,
                                    h=NH)
            tt(out=xlg5, in0=xlg5, in1=wb, op=OP.mult)

            comb = small.tile([128, 4, F + NH], F32, tag=tg + "_comb")
            red(out=comb[:, :, 0:F],
                in_=xlg[:].rearrange("p (t s f) -> p t f s", t=4, s=S1),
                axis=AX.X, op=OP.add)
            red(out=comb[:, :, F:F + NH],
                in_=lg[:].rearrange("p t s h -> p t h s"),
                axis=AX.X, op=OP.add)

            nd_ps = psum_acc.tile([128, F + NH], F32, tag=tg + "_ndps")
            cmb1 = cv("CMB1")
            for t in range(4):
                nc.tensor.matmul(nd_ps[:],
                                 cmb1[:, 128 * (4 * h + t):
                                      128 * (4 * h + t + 1)],
                                 comb[:, t, :],
                                 start=(t == 0), stop=(t == 3))
            nd = small.tile([128, F + NH], F32, tag=tg + "_nd")
            nc.vector.tensor_copy(out=nd[:], in_=nd_ps[:])
            dma(out=D["T1"][128 * h:128 * (h + 1), F:2 * F + NH], in_=nd[:])
            g1b = head_mean_bias_elu(nd[:], 128, cv("g1bias"), tg + "_g")
            g1b_chunks.append(g1b)

        # g1_base^T -> XL2_base (T2 rows 0:256)
        g1bT = small.tile([64, 256], F32, tag="g1bT")
        for h in range(2):
            ps = psum.tile([64, 128], F32, tag="ps")
            nc.tensor.transpose(ps[:], g1b_chunks[h], ident[:])
            nc.vector.tensor_copy(out=g1bT[:, 128 * h:128 * (h + 1)],
                                  in_=ps[:])
        for h in range(2):
            ps = psum.tile([128, 128], F32, tag="ps")
            nc.tensor.matmul(ps[:], g1bT[:, 128 * h:128 * (h + 1)],
                             cv("g2_wl"), start=True, stop=True)
            sb = small.tile([128, 128], F32, tag="p15_sb%d" % h)
            nc.vector.tensor_copy(out=sb[:], in_=ps[:])
            dma(out=D["T2"][128 * h:128 * (h + 1), :], in_=sb[:])

        # ---------------- (b): full recompute of dst v ----------------
        lgb = edge_stage(xlgb, SB, cv("MSKBD"), att1, xrrb, "b")
        combb = softmax_combine(xlgb, lgb, SB, "b")
        ndb_ps = psum.tile([VPC, F + NH], F32, tag="ps")
        nc.tensor.matmul(ndb_ps[:], cv("CMBBD"), combb[:],
                         start=True, stop=True)
        ndb = small.tile([VPC, F + NH], F32, tag="b_nd")
        nc.vector.tensor_copy(out=ndb[:], in_=ndb_ps[:])
        g1self = head_mean_bias_elu(ndb[:], VPC, cv("g1bias"), "bg1")

        ps_t = psum.tile([C2, VPC], F32, tag="ps")
        nc.tensor.transpose(ps_t[:], g1self, ident[:VPC, :VPC])
        g1sT = small.tile([C2, VPC], F32, tag="g1sT")
        nc.vector.tensor_copy(out=g1sT[:], in_=ps_t[:])
        ps_l = psum.tile([VPC, F], F32, tag="ps")
        nc.tensor.matmul(ps_l[:], g1sT[:], cv("g2_wl"), start=True, stop=True)
        sb_l = small.tile([VPC, F], F32, tag="b_sbl")
        nc.vector.tensor_copy(out=sb_l[:], in_=ps_l[:])
        dma(out=D["T2"][N:N + VPC, :], in_=sb_l[:])
        ps_r = psum.tile([VPC, F], F32, tag="ps")
        nc.tensor.matmul(ps_r[:], g1sT[:], cv("g2_wr"), start=True, stop=True)
        sb_r = small.tile([VPC, F], F32, tag="b_sbr")
        tt(out=sb_r[:], in0=ps_r[:], in1=cv("blr")[:VPC, :], op=OP.add)
        dma(out=D["XR2S"][:], in_=sb_r[:])

        # ---------------- (a): light dst updates ----------------
        t1g = big.tile([128, KA * 320], F32, tag="a_t1g")
        dgather(t1g[:].rearrange("p (k f) -> p k f", k=KA),
                D["T1"][:], iv16("IDX_A_T1W"), KA * 128, 320)

        t1g3 = t1g[:].rearrange("p (k f) -> p k f", k=KA)  # f = 320
        t1xr = t1g3[:, :, 0:F]
        t1num = t1g3[:, :, F:2 * F]
        t1den = t1g3[:, :, 2 * F:2 * F + NH]
        xlv3 = xlv[:].rearrange("p (k f) -> p k f", k=KA)
        xlsv3 = xlsv[:].rearrange("p (k f) -> p k f", k=KA)
        ca = cv("C_A")

        def logits_expC(xl3, tg):
            u = big.tile([128, KA * F], F32, tag="a_u" + tg)
            u3 = u[:].rearrange("p (k f) -> p k f", k=KA)
            tt(out=u3, in0=xl3, in1=t1xr, op=OP.add)
            lrelu(u[:], KA * F, "a_u" + tg)
            attb = att1.rearrange("p (h f) -> p () h f", h=NH) \
                .to_broadcast([128, KA, NH, C2])
            u4 = u[:].rearrange("p (k h f) -> p k h f", k=KA, h=NH)
            tt(out=u4, in0=u4, in1=attb, op=OP.mult)
            lw = small.tile([128, KA, NH], F32, tag="a_lw" + tg)
            red(out=lw[:], in_=u4, axis=AX.X, op=OP.add)
            act(out=lw[:], in_=lw[:], func=AF.Exp)
            cb = ca.rearrange("p k -> p k ()").to_broadcast([128, KA, NH])
            tt(out=lw[:], in0=lw[:], in1=cb, op=OP.mult)
            return lw

        wn = logits_expC(xlsv3, "n")    # C * w_new
        wo = logits_expC(xlv3, "o")     # C * w_old

        dden = small.tile([128, KA, NH], F32, tag="a_dden")
        tt(out=dden[:], in0=wn[:], in1=wo[:], op=OP.subtract)
        tt(out=dden[:], in0=dden[:], in1=t1den, op=OP.add)
        wnb = wn[:].rearrange("p k h -> p k h ()") \
            .to_broadcast([128, KA, NH, C2])
        xlsv4 = xlsv[:].rearrange("p (k h f) -> p k h f", k=KA, h=NH)
        tt(out=xlsv4, in0=xlsv4, in1=wnb, op=OP.mult)
        wob = wo[:].rearrange("p k h -> p k h ()") \
            .to_broadcast([128, KA, NH, C2])
        xlv4 = xlv[:].rearrange("p (k h f) -> p k h f", k=KA, h=NH)
        tt(out=xlv4, in0=xlv4, in1=wob, op=OP.mult)
        tt(out=xlsv3, in0=xlsv3, in1=xlv3, op=OP.subtract)
        tt(out=xlsv3, in0=xlsv3, in1=t1num, op=OP.add)
        nc.vector.reciprocal(out=dden[:], in_=dden[:])
        ddb = dden[:].rearrange("p k h -> p k h ()") \
            .to_broadcast([128, KA, NH, C2])
        tt(out=xlsv4, in0=xlsv4, in1=ddb, op=OP.mult)
        radd = big.tile([128, KA, C2], F32, tag="a_radd")
        tt(out=radd[:], in0=xlsv4[:, :, 0, :], in1=xlsv4[:, :, 1, :],
           op=OP.add)
        ts_mul(radd[:], radd[:], 0.5)
        g1bb = cv("g1bias").rearrange("p f -> p () f").to_broadcast(
            [128, KA, C2])
        tt(out=radd[:], in0=radd[:], in1=g1bb, op=OP.add)
        radd_flat = radd[:].rearrange("p k f -> p (k f)")
        elu_inplace(radd_flat, big, KA * C2, "a_elu")
        dma(out=D["G1L"][:].rearrange("(p k) f -> p k f", p=128), in_=radd[:])

        # ---------------- rare light rows -> T2 rows 288: ----------------
        grare = small.tile([128, C2], F32, tag="r_g")
        gather(out=grare[:], out_offset=None, in_=D["G1L"][:],
               in_offset=IOA(ap=iv32("IDX_RARE"), axis=0))
        ps_rt = psum.tile([C2, 128], F32, tag="ps")
        nc.tensor.transpose(ps_rt[:], grare[:], ident[:])
        grT = small.tile([C2, 128], F32, tag="grT")
        nc.vector.tensor_copy(out=grT[:], in_=ps_rt[:])
        ps_rm = psum.tile([128, F], F32, tag="ps")
        nc.tensor.matmul(ps_rm[:], grT[:], cv("g2_wl"), start=True, stop=True)
        sb_rm = small.tile([128, F], F32, tag="r_sb")
        nc.vector.tensor_copy(out=sb_rm[:], in_=ps_rm[:])
        dma(out=D["T2"][N + VPC:N + VPC + 128, :], in_=sb_rm[:])

        # ---------------- (d): layer 2 at dst v ----------------
        xl2g = big.tile([128, SB * F], F32, tag="d_xlg")
        dgather(xl2g[:].rearrange("p (k f) -> p k f", k=SB),
                D["T2"][:], iv16("IDX_DW"), SB * 128, F)
        xr2r = big.tile([128, F], F32, tag="d_xrr")
        gather(out=xr2r[:], out_offset=None, in_=D["XR2S"][:],
               in_offset=IOA(ap=iv32("IDX_D_V"), axis=0))
        lgd = edge_stage(xl2g, SB, cv("MSKBD"), cv("att2"), xr2r, "d")
        combd = softmax_combine(xl2g, lgd, SB, "d")
        ndd_ps = psum.tile([VPC, F + NH], F32, tag="ps")
        nc.tensor.matmul(ndd_ps[:], cv("CMBBD"), combd[:],
                         start=True, stop=True)
        ndd = small.tile([VPC, F + NH], F32, tag="d_nd")
        nc.vector.tensor_copy(out=ndd[:], in_=ndd_ps[:])
        g2row = head_mean_bias_elu(ndd[:], VPC, cv("g2bias"), "dg2")

        # out = tanh(g2row @ rec_w + rec_b)
        ps_ot = psum.tile([C2, VPC], F32, tag="ps")
        nc.tensor.transpose(ps_ot[:], g2row, ident[:VPC, :VPC])
        g2T = small.tile([C2, VPC], F32, tag="g2T")
        nc.vector.tensor_copy(out=g2T[:], in_=ps_ot[:])
        ps_om = psum.tile([C2, VPC], F32, tag="ps")
        nc.tensor.matmul(ps_om[:], cv("rec_w"), g2T[:], start=True,
                         stop=True)
        outT = small.tile([C2, VPC], F32, tag="outT")
        act(out=outT[:], in_=ps_om[:], func=AF.Tanh, bias=cv("rec_b"))
        ps_of = psum.tile([VPC, C2], F32, tag="ps")
        nc.tensor.transpose(ps_of[:], outT[:], ident[:C2, :C2])
        outsb = small.tile([VPC, C2], F32, tag="outsb")
        nc.vector.tensor_copy(out=outsb[:], in_=ps_of[:])
        dma(out=D["out"][:], in_=outsb[:])


# --------------------------------------------------------------------------
# Entry point
# --------------------------------------------------------------------------

def _make_in_maps(inputs, shared, percore, dims):
    f32 = np.float32
    (off_f, nf), (off_i16, ni16), (off_i32, ni32) = _pack_layout(dims)

    def rep(v):
        a = np.asarray(v, f32).reshape(1, -1)
        return np.ascontiguousarray(np.broadcast_to(a, (128, a.shape[1])))

    vals = {
        "conv_b": np.asarray(inputs["conv_b"], f32).reshape(128, 1),
        "lin2_b": np.asarray(inputs["lin2_b"], f32).reshape(64, 1),
        "g1_bl": np.asarray(inputs["g1_bl"], f32).reshape(128, 1),
        "g1_br": np.asarray(inputs["g1_br"], f32).reshape(128, 1),
        "rec_b": np.asarray(inputs["rec_b"], f32).reshape(64, 1),
        "att1": rep(inputs["g1_att"]),
        "att2": rep(inputs["g2_att"]),
        "g1bias": rep(inputs["g1_bias"]),
        "g2bias": rep(inputs["g2_bias"]),
        "blr": rep(inputs["g2_bl"] + inputs["g2_br"]),
        "CMB1": shared["CMB1"].transpose(1, 0, 2, 3).reshape(128, -1),
        "MSK1": shared["MSK1"].transpose(1, 0, 2).reshape(128, -1),
    }
    for nm in ("node_proj", "emb_proj", "conv_w0", "conv_w1", "lin2_w",
               "masked_proj", "normal_proj", "g1_wl", "g1_wr", "g2_wl",
               "g2_wr", "rec_w"):
        vals[nm] = np.asarray(inputs[nm], f32)

    x = np.asarray(inputs["x"], f32)
    E = np.asarray(inputs["E_emb"], f32)
    xE = np.concatenate([x[0:128], x[128:256], E[0:128], E[128:256]],
                        axis=1)

    def fill(off_map, total, npdtype, core_vals):
        out = np.zeros((128, total), npdtype)
        for nm, (o, rows, cols) in off_map.items():
            a = core_vals[nm]
            assert a.shape == (rows, cols) or a.shape[0] <= rows, \
                (nm, a.shape, rows, cols)
            out[:a.shape[0], o:o + cols] = a
        return out

    in_maps = []
    for c in range(NCORES):
        t = percore[c]
        cvals = dict(vals)
        cvals["CMBBD"] = t["CMBBD"]
        cvals["MSKBD"] = t["MSKBD"]
        cvals["C_A"] = t["C_A"]
        i16vals = {"IDX1W0": shared["IDX1W"][0],
                   "IDX1W1": shared["IDX1W"][1],
                   "IDX_BW": t["IDX_BW"], "IDX_DW": t["IDX_DW"],
                   "IDX_A_T1W": t["IDX_A_T1W"], "IDX_A_VW": t["IDX_A_VW"],
                   "IDX_A_VSW": t["IDX_A_VSW"]}
        i32vals = {"IDXD1": shared["IDXD1"].transpose(1, 0, 2)
                   .reshape(128, 8),
                   "IDX_B_V": t["IDX_B_V"], "IDX_D_V": t["IDX_D_V"],
                   "IDX_RARE": t["IDX_RARE"]}
        in_maps.append({
            "packf": fill(off_f, nf, np.float32, cvals),
            "packi16": fill(off_i16, ni16, np.int16, i16vals),
            "packi32": fill(off_i32, ni32, np.int32, i32vals),
            "xE": np.ascontiguousarray(xE),
        })
    return in_maps


_CACHE = {}
TRACE = False          # set by test.py to capture NTFF profiles
LAST_RESULT = None


def kernel(**inputs):
    global LAST_RESULT
    inputs = {k: np.asarray(v) for k, v in inputs.items()}
    shared, percore, dims = _build_tables(inputs["edge_index"])
    key = (dims["S1"], dims["SB"], dims["K2"])
    if key not in _CACHE:
        _CACHE[key] = _build_program(dims)
    nc = _CACHE[key]
    in_maps = _make_in_maps(inputs, shared, percore, dims)
    kw = {}
    if TRACE:
        kw = dict(trace=True, trace_cores=list(range(NCORES)))
    res = run_bass_kernel_spmd(nc, in_maps, core_ids=list(range(NCORES)),
                               **kw)
    LAST_RESULT = res
    out = np.concatenate([res.results[c]["out"] for c in range(NCORES)],
                         axis=0)
    return out.astype(np.float32)


# revision 21
# speedup vs baseline: 1.7031x; 1.0700x over previous
"""Trainium2 Bass kernel for nn_NodeDetector (masked-node GATv2 ensemble).

Algorithm: the reference vmaps a full 2-layer GATv2 over 256 "masked node"
variants, but variant v differs from the shared base computation in exactly
one input row (row v).  We compute the base graph once and apply sparse
incremental updates per variant:

  phase 0  dense projections -> XL/XR (base rows) and XLs/XRs (masked rows)
  phase 1  base GAT layer 1: per-dst softmax sums (num1/den1) + g1_base
  (a)      per variant v: "light" g1 updates at out-neighbors d of v
           (only edges v->d changed: closed-form num/den delta)
  (b)      per variant v: full recompute of g1 at node v
  (d)      layer 2 at dst v only: gather xl2 of in-neighbors (base / self /
           rare light rows), one softmax, project + tanh.

Attention softmaxes skip the per-dst max subtraction (mathematically
identical; logits are O(10) so fp32 exp is safe).  All gathers use
host-built index tables (edge_index is host data) via gpsimd indirect DMA.
Work is sharded 32 variants per core across 8 cores; phases 0/1 are
replicated per core.  No collectives.

Perf notes vs the first working version:
  - edge slots are degree-aware bin-packed (variable rows per dst) instead
    of a fixed ceil(max_deg/4) per row: ~40% less gather + DVE volume.
  - all constants/index tables ship in 4 packed DMAs (dma_start issue on
    the sync queue costs ~600ns each; the old code had ~35 of them).
  - per-half p1 tiles get distinct tags so half 1's gather is not
    WAR-serialized against half 0's vector reads.
  - node-major XL/XR tables are stored with one strided DMA each.
"""

import numpy as np

import concourse.bass as bass
import concourse.mybir as mybir
import concourse.tile as tile
from concourse import bacc
from concourse.bass_utils import run_bass_kernel_spmd
from concourse.masks import make_identity

F32 = mybir.dt.float32
BF16 = mybir.dt.bfloat16
I32 = mybir.dt.int32
I16 = mybir.dt.int16
AF = mybir.ActivationFunctionType
OP = mybir.AluOpType
AX = mybir.AxisListType

N = 256          # nodes / variants
F = 128          # NUM_HEAD * C2
C2 = 64
NH = 2
NCORES = 8
VPC = N // NCORES   # variants per core = 32
NEG = 0.2           # leaky relu slope


# --------------------------------------------------------------------------
# Host-side table construction
# --------------------------------------------------------------------------

def _wrap16(flat):
    """int16 idx layout for dma_gather: value for flat position i lives
    at [i % 16, i // 16], tiled to 128 partitions."""
    flat = np.asarray(flat)
    num = flat.shape[0]
    A = np.zeros((16, num // 16), np.int16)
    A[np.arange(num) % 16, np.arange(num) // 16] = flat.astype(np.int16)
    return np.ascontiguousarray(np.tile(A, (8, 1)))


def _wrapPK(idx_pk):
    """[128, K] logical idx (out[p, k] = tab[idx_pk[p,k]]) -> wrapped."""
    return _wrap16(idx_pk.T.reshape(-1))


def _min_slots(degs, nrows):
    S = 1
    while sum(-(-d // S) for d in degs) > nrows:
        S += 1
    return S


def _binpack(dst_list, in_edges, nrows, S):
    """Rows of (dst, edge_id_chunk), <= S edges each, padded to nrows."""
    rows = []
    for d in dst_list:
        el = in_edges[d]
        for i in range(0, len(el), S):
            rows.append((d, el[i:i + S]))
    assert len(rows) <= nrows
    rows += [None] * (nrows - len(rows))
    return rows


def _build_tables(edge_index):
    src = edge_index[0].astype(np.int64)
    dst = edge_index[1].astype(np.int64)
    E = src.shape[0]

    in_edges = [[] for _ in range(N)]
    for e in range(E):
        in_edges[dst[e]].append(e)

    # p1: two halves of 128 dst, 512 rows each, degree-aware slot packing
    S1 = max(_min_slots([len(in_edges[d]) for d in range(128 * h,
                                                         128 * (h + 1))], 512)
             for h in range(2))
    # b/d: per-core 32 dst over 128 rows; one global slot count
    SB = max(_min_slots([len(in_edges[v]) for v in range(VPC * c,
                                                         VPC * (c + 1))], 128)
             for c in range(NCORES))

    out_by_src = [[] for _ in range(N)]
    for e in range(E):
        if dst[e] != src[e]:
            out_by_src[src[e]].append(int(dst[e]))
    light = []
    for v in range(N):
        cnt = {}
        for d in out_by_src[v]:
            cnt[d] = cnt.get(d, 0) + 1
        light.append(sorted(cnt.items()))
    max_light = max(len(l) for l in light)
    K2 = 4 * (-(-max_light // 4))       # light slots per variant, mult of 4
    KA = K2 * VPC // 128                # light slots per partition

    shared = {}
    IDX1W = np.zeros((2, 128, 4 * S1 * 8), np.int16)
    MSK1 = np.zeros((2, 128, 4 * S1), np.float32)
    IDXD1 = np.zeros((2, 128, 4), np.int32)
    CMB1 = np.zeros((2, 128, 4, 128), np.float32)
    for h in range(2):
        rows = _binpack(range(128 * h, 128 * (h + 1)), in_edges, 512, S1)
        pk = np.zeros((128, 4 * S1), np.int64)
        for r, ent in enumerate(rows):
            t, p = divmod(r, 128)
            if ent is None:
                continue
            d, el = ent
            IDXD1[h, p, t] = d
            CMB1[h, p, t, d - 128 * h] = 1.0
            for si, e in enumerate(el):
                pk[p, t * S1 + si] = src[e]
                MSK1[h, p, t * S1 + si] = 1.0
        IDX1W[h] = _wrapPK(pk)
    shared["IDX1W"] = IDX1W
    shared["MSK1"] = MSK1
    shared["IDXD1"] = IDXD1
    shared["CMB1"] = CMB1

    percore = []
    for c in range(NCORES):
        t = {}
        V = list(range(c * VPC, (c + 1) * VPC))

        rows = _binpack(V, in_edges, 128, SB)
        IDXB = np.zeros((128, SB), np.int64)
        IDXD2 = np.zeros((128, SB), np.int64)
        MSKBD = np.zeros((128, SB), np.float32)
        CMBBD = np.zeros((128, VPC), np.float32)
        IDX_B_V = np.zeros((128, 1), np.int32)
        IDX_D_V = np.zeros((128, 1), np.int32)
        IDX_RARE = np.zeros((128, 1), np.int32)
        rare_map = {}
        for r, ent in enumerate(rows):
            if ent is None:
                continue
            v, el = ent
            vi = v - c * VPC
            CMBBD[r, vi] = 1.0
            IDX_B_V[r, 0] = 256 + v     # XRcat self half
            IDX_D_V[r, 0] = vi          # XR2S row
            lpos = {d: i for i, (d, _) in enumerate(light[v])}
            for si, e in enumerate(el):
                sn = int(src[e])
                MSKBD[r, si] = 1.0
                IDXB[r, si] = 256 + v if sn == v else sn
                if sn == v:
                    IDXD2[r, si] = 256 + vi
                elif sn in lpos:
                    key = (vi, sn)
                    if key not in rare_map:
                        rs = len(rare_map)
                        assert rs < 128, "rare-row overflow"
                        rare_map[key] = rs
                        IDX_RARE[rs, 0] = vi * K2 + lpos[sn]
                    IDXD2[r, si] = 288 + rare_map[key]
                else:
                    IDXD2[r, si] = sn
        t["IDX_BW"] = _wrapPK(IDXB)
        t["IDX_DW"] = _wrapPK(IDXD2)
        t["MSKBD"] = MSKBD
        t["CMBBD"] = CMBBD
        t["IDX_B_V"] = IDX_B_V
        t["IDX_D_V"] = IDX_D_V
        t["IDX_RARE"] = IDX_RARE

        IDX_A_T1 = np.zeros((128, KA), np.int64)
        IDX_A_V = np.zeros((128, KA), np.int64)
        IDX_A_VS = np.zeros((128, KA), np.int64)
        C_A = np.zeros((128, KA), np.float32)
        for r in range(128 * KA):
            vi, slot = divmod(r, K2)
            p, k = divmod(r, KA)
            v = V[vi]
            IDX_A_V[p, k] = v
            IDX_A_VS[p, k] = 256 + v
            if slot < len(light[v]):
                d, cc = light[v][slot]
                IDX_A_T1[p, k] = d
                C_A[p, k] = float(cc)
        t["IDX_A_T1W"] = _wrapPK(IDX_A_T1)
        t["IDX_A_VW"] = _wrapPK(IDX_A_V)
        t["IDX_A_VSW"] = _wrapPK(IDX_A_VS)
        t["C_A"] = C_A
        percore.append(t)

    dims = dict(S1=S1, SB=SB, K2=K2, KA=KA)
    return shared, percore, dims


# --------------------------------------------------------------------------
# Packed-input layouts (single source of truth for device + host)
# --------------------------------------------------------------------------

def _pack_layout(dims):
    S1, SB, KA = dims["S1"], dims["SB"], dims["KA"]
    pf = [("node_proj", 64, 128), ("emb_proj", 64, 128),
          ("conv_w0", 128, 128), ("conv_w1", 128, 128), ("conv_b", 128, 1),
          ("lin2_w", 128, 64), ("lin2_b", 64, 1), ("masked_proj", 64, 64),
          ("normal_proj", 64, 64), ("g1_wl", 64, 128), ("g1_bl", 128, 1),
          ("g1_wr", 64, 128), ("g1_br", 128, 1), ("g2_wl", 64, 128),
          ("g2_wr", 64, 128), ("rec_w", 64, 64), ("rec_b", 64, 1),
          ("att1", 128, 128), ("att2", 128, 128), ("g1bias", 128, 64),
          ("g2bias", 128, 64), ("blr", 128, 128),
          ("CMB1", 128, 8 * 128), ("CMBBD", 128, VPC),
          ("MSK1", 128, 8 * S1), ("MSKBD", 128, SB), ("C_A", 128, KA)]
    pi16 = [("IDX1W0", 128, 4 * S1 * 8), ("IDX1W1", 128, 4 * S1 * 8),
            ("IDX_BW", 128, SB * 8), ("IDX_DW", 128, SB * 8),
            ("IDX_A_T1W", 128, KA * 8), ("IDX_A_VW", 128, KA * 8),
            ("IDX_A_VSW", 128, KA * 8)]
    pi32 = [("IDXD1", 128, 8), ("IDX_B_V", 128, 1), ("IDX_D_V", 128, 1),
            ("IDX_RARE", 128, 1)]

    def offsets(lst):
        off, o = {}, 0
        for nm, rows, cols in lst:
            off[nm] = (o, rows, cols)
            o += cols
        return off, o

    return offsets(pf), offsets(pi16), offsets(pi32)


# --------------------------------------------------------------------------
# Device program
# --------------------------------------------------------------------------

def _build_program(dims):
    S1, SB, K2, KA = dims["S1"], dims["SB"], dims["K2"], dims["KA"]
    (off_f, nf), (off_i16, ni16), (off_i32, ni32) = _pack_layout(dims)

    nc = bacc.Bacc("TRN2", target_bir_lowering=False, debug=False)

    D = {}
    D["packf"] = nc.dram_tensor("packf", [128, nf], F32,
                                kind="ExternalInput")
    D["packi16"] = nc.dram_tensor("packi16", [128, ni16], I16,
                                  kind="ExternalInput")
    D["packi32"] = nc.dram_tensor("packi32", [128, ni32], I32,
                                  kind="ExternalInput")
    D["xE"] = nc.dram_tensor("xE", [128, 256], F32, kind="ExternalInput")

    D["out"] = nc.dram_tensor("out", [VPC, 64], F32, kind="ExternalOutput")
    D["XLcat"] = nc.dram_tensor("XLcat", [2 * N, F], BF16)
    D["XRcat"] = nc.dram_tensor("XRcat", [2 * N, F], BF16)
    D["T1"] = nc.dram_tensor("T1", [N, 192], F32)   # row 768B (gather needs %256B); cols 130:192 unused
    D["G1L"] = nc.dram_tensor("G1L", [VPC * K2, C2], F32)
    D["T2"] = nc.dram_tensor("T2", [N + VPC + 128, F], BF16)
    D["XR2S"] = nc.dram_tensor("XR2S", [VPC, F], BF16)

    with tile.TileContext(nc) as tc:
        _trace(nc, tc, D, dims, (off_f, off_i16, off_i32))
    nc.compile()
    return nc


def _trace(nc, tc, D, dims, offs):
    S1, SB, K2, KA = dims["S1"], dims["SB"], dims["K2"], dims["KA"]
    off_f, off_i16, off_i32 = offs
    import contextlib
    ctx = contextlib.ExitStack()
    with ctx:
        consts = ctx.enter_context(tc.tile_pool(name="consts", bufs=1))
        small = ctx.enter_context(tc.tile_pool(name="small", bufs=1))
        big = ctx.enter_context(tc.tile_pool(name="big", bufs=1))
        psum = ctx.enter_context(tc.tile_pool(name="psum", bufs=4,
                                              space="PSUM"))
        psum_acc = ctx.enter_context(tc.tile_pool(name="psacc", bufs=2,
                                                  space="PSUM"))

        dma = nc.sync.dma_start

        def dgather(out_ap, in_ap, idx_ap, num, elem):
            nc.gpsimd.dma_gather(out_ap=out_ap, in_ap=in_ap, idxs_ap=idx_ap,
                                 num_idxs=num, num_idxs_reg=num,
                                 elem_size=elem, single_packet=False)
        tt = nc.vector.tensor_tensor
        red = nc.vector.tensor_reduce
        act = nc.scalar.activation
        gather = nc.gpsimd.indirect_dma_start
        IOA = bass.IndirectOffsetOnAxis

        # ---------------- constants (4 packed DMAs) ----------------
        ident = consts.tile([128, 128], F32, tag="ident")
        make_identity(nc, ident[:])

        xE = consts.tile([128, 256], F32, tag="xE")
        dma(out=xE[:], in_=D["xE"][:])
        packf = consts.tile([128, D["packf"].shape[1]], F32, tag="packf")
        dma(out=packf[:], in_=D["packf"][:])
        packi16 = consts.tile([128, D["packi16"].shape[1]], I16,
                              tag="packi16")
        dma(out=packi16[:], in_=D["packi16"][:])
        packi32 = consts.tile([128, D["packi32"].shape[1]], I32,
                              tag="packi32")
        dma(out=packi32[:], in_=D["packi32"][:])

        def cv(name):
            o, rows, cols = off_f[name]
            return packf[:rows, o:o + cols]

        def iv16(name):
            o, rows, cols = off_i16[name]
            return packi16[:rows, o:o + cols]

        def iv32(name):
            o, rows, cols = off_i32[name]
            return packi32[:rows, o:o + cols]

        # bf16 copies of the edge-pipeline constants
        def bfcast(name):
            o, rows, cols = off_f[name]
            t_ = consts.tile([rows, cols], BF16, tag="bf_" + name)
            nc.vector.tensor_copy(out=t_[:], in_=packf[:rows, o:o + cols])
            return t_[:]

        # ---------------- helpers ----------------
        def ts_mul(out, in0, s):
            nc.vector.tensor_scalar_mul(out=out, in0=in0, scalar1=s)

        def lrelu(flat_ap, nfree, tag, dt=F32):
            t_ = big.tile([128, nfree], dt, tag="lr_" + tag)
            ta = t_[:flat_ap.shape[0], :]
            ts_mul(ta, flat_ap, NEG)
            tt(out=flat_ap, in0=flat_ap, in1=ta, op=OP.max)

        def elu_inplace(x_ap, scratch_pool, nfree, tag):
            xpos = scratch_pool.tile([128, nfree], F32, tag=tag + "_xp")
            nrow = x_ap.shape[0]
            xp = xpos[:nrow, :]
            nc.vector.tensor_scalar_max(out=xp, in0=x_ap, scalar1=0.0)
            nc.vector.tensor_scalar_min(out=x_ap, in0=x_ap, scalar1=0.0)
            act(out=x_ap, in_=x_ap, func=AF.Exp)
            nc.vector.tensor_scalar_add(out=x_ap, in0=x_ap, scalar1=-1.0)
            nc.vector.tensor_add(out=x_ap, in0=x_ap, in1=xp)
            return x_ap

        def head_mean_bias_elu(nd_ap, nrow, bias_rep, tag):
            """nd_ap [nrow, F+NH] = (num|den) -> elu(mean_h(num/den)+bias)."""
            rec = small.tile([128, NH], F32, tag=tag + "_rec")
            nc.vector.reciprocal(out=rec[:nrow, :], in_=nd_ap[:, F:F + NH])
            r0 = small.tile([128, C2], F32, tag=tag + "_r0")
            r1 = small.tile([128, C2], F32, tag=tag + "_r1")
            ts_mul(r0[:nrow, :], nd_ap[:, 0:C2], rec[:nrow, 0:1])
            ts_mul(r1[:nrow, :], nd_ap[:, C2:F], rec[:nrow, 1:2])
            tt(out=r0[:nrow, :], in0=r0[:nrow, :], in1=r1[:nrow, :], op=OP.add)
            ts_mul(r0[:nrow, :], r0[:nrow, :], 0.5)
            tt(out=r0[:nrow, :], in0=r0[:nrow, :], in1=bias_rep[:nrow, :],
               op=OP.add)
            return elu_inplace(r0[:nrow, :], small, C2, tag)

        # ---------------- phase 0 ----------------
        def mm_to_sbuf(lhsT, rhs, M, Nf, tag, bias=None, func=AF.Identity,
                       extra=None):
            out_tile = small.tile([M, Nf], F32, tag=tag)
            ps = psum.tile([128, 256], F32, tag="ps")
            nc.tensor.matmul(ps[:M, :Nf], lhsT, rhs, start=True,
                             stop=extra is None)
            if extra is not None:
                nc.tensor.matmul(ps[:M, :Nf], extra[0], extra[1],
                                 start=False, stop=True)
            if bias is None:
                act(out=out_tile[:], in_=ps[:M, :Nf], func=func)
            else:
                act(out=out_tile[:], in_=ps[:M, :Nf], func=func, bias=bias)
            return out_tile

        xT = small.tile([64, 256], F32, tag="xT")
        eT = small.tile([64, 256], F32, tag="eT")
        for h in range(2):
            for (col0, dstT) in ((64 * h, xT), (128 + 64 * h, eT)):
                pst = psum.tile([64, 128], F32, tag="ps")
                nc.tensor.transpose(pst[:], xE[:, col0:col0 + 64], ident[:])
                nc.vector.tensor_copy(out=dstT[:, 128 * h:128 * (h + 1)],
                                      in_=pst[:])

        xpT = mm_to_sbuf(cv("node_proj"), xT[:], 128, 256, "xpT")
        epT = mm_to_sbuf(cv("emb_proj"), eT[:], 128, 256, "epT")
        HbT = mm_to_sbuf(cv("conv_w0"), epT[:], 128, 256, "HbT",
                         bias=cv("conv_b"), func=AF.Tanh,
                         extra=(cv("conv_w1"), xpT[:]))
        HsT = mm_to_sbuf(cv("conv_w0"), epT[:], 128, 256, "HsT",
                         bias=cv("conv_b"), func=AF.Tanh)
        MbT = mm_to_sbuf(cv("lin2_w"), HbT[:], 64, 256, "MbT",
                         bias=cv("lin2_b"))
        MsT = mm_to_sbuf(cv("lin2_w"), HsT[:], 64, 256, "MsT",
                         bias=cv("lin2_b"))
        PbT = mm_to_sbuf(cv("normal_proj"), MbT[:], 64, 256, "PbT")
        PsT = mm_to_sbuf(cv("masked_proj"), MsT[:], 64, 256, "PsT")
        XLT = mm_to_sbuf(cv("g1_wl"), PbT[:], 128, 256, "XLT",
                         bias=cv("g1_bl"))
        XRT = mm_to_sbuf(cv("g1_wr"), PbT[:], 128, 256, "XRT",
                         bias=cv("g1_br"))
        XLsT = mm_to_sbuf(cv("g1_wl"), PsT[:], 128, 256, "XLsT",
                          bias=cv("g1_bl"))
        XRsT = mm_to_sbuf(cv("g1_wr"), PsT[:], 128, 256, "XRsT",
                          bias=cv("g1_br"))

        # node-major tables (bf16), contiguous per-chunk DMAs
        def cat_store(srcTs, dram):
            sb = small.tile([128, 4 * 128], BF16, tag="cat_" + dram.name)
            for k, (srcT, h) in enumerate(srcTs):
                ps = psum.tile([128, 128], F32, tag="ps")
                nc.tensor.transpose(ps[:], srcT[:, 128 * h:128 * (h + 1)],
                                    ident[:])
                nc.vector.tensor_copy(out=sb[:, 128 * k:128 * (k + 1)],
                                      in_=ps[:])
                dma(out=dram[128 * k:128 * (k + 1), :],
                    in_=sb[:, 128 * k:128 * (k + 1)])
            return sb

        cat_store([(XLT, 0), (XLT, 1), (XLsT, 0), (XLsT, 1)], D["XLcat"])
        cat_store([(XRT, 0), (XRT, 1), (XRsT, 0), (XRsT, 1)], D["XRcat"])

        # ---------------- early gathers (gpsimd queue order) -------------
        xlg1 = []
        xrr1 = []
        for h in range(2):
            xlg = big.tile([128, 4 * S1 * F], BF16, tag="p1_xlg%d" % h)
            dgather(xlg[:].rearrange("p (k f) -> p k f", k=4 * S1),
                    D["XLcat"][:], iv16("IDX1W%d" % h), 4 * S1 * 128, F)
            xrr = big.tile([128, 4, F], BF16, tag="p1_xrr%d" % h)
            for t in range(4):
                gather(out=xrr[:, t, :], out_offset=None, in_=D["XRcat"][:],
                       in_offset=IOA(ap=iv32("IDXD1")[:, 4 * h + t:
                                                      4 * h + t + 1],
                                     axis=0))
            xlg1.append(xlg)
            xrr1.append(xrr)

        xlgb = big.tile([128, SB * F], BF16, tag="b_xlg")
        dgather(xlgb[:].rearrange("p (k f) -> p k f", k=SB),
                D["XLcat"][:], iv16("IDX_BW"), SB * 128, F)
        xrrb = big.tile([128, F], BF16, tag="b_xrr")
        gather(out=xrrb[:], out_offset=None, in_=D["XRcat"][:],
               in_offset=IOA(ap=iv32("IDX_B_V"), axis=0))

        xlv = big.tile([128, KA * F], BF16, tag="a_xlv")
        dgather(xlv[:].rearrange("p (k f) -> p k f", k=KA),
                D["XLcat"][:], iv16("IDX_A_VW"), KA * 128, F)
        xlsv = big.tile([128, KA * F], BF16, tag="a_xlsv")
        dgather(xlsv[:].rearrange("p (k f) -> p k f", k=KA),
                D["XLcat"][:], iv16("IDX_A_VSW"), KA * 128, F)
        t1xrg = big.tile([128, KA * F], BF16, tag="a_t1xr")
        dgather(t1xrg[:].rearrange("p (k f) -> p k f", k=KA),
                D["XRcat"][:], iv16("IDX_A_T1W"), KA * 128, F)

        # ---------------- shared GAT edge stage ----------------
        def edge_stage(xlg_tile, nslot, mask_ap, att, xr_tile, tagp):
            """xlg_tile [128, nslot*F] gathered xl rows (consumed -> w*xl).
            xr_tile [128, F]; returns w tile [128, nslot, NH]."""
            xlg3 = xlg_tile[:].rearrange("p (s f) -> p s f", s=nslot)
            u = big.tile([128, nslot * F], BF16, tag=tagp + "_u")
            u3 = u[:].rearrange("p (s f) -> p s f", s=nslot)
            tt(out=u3, in0=xlg3,
               in1=xr_tile[:].rearrange("p f -> p () f")
               .to_broadcast([128, nslot, F]), op=OP.add)
            lrelu(u[:], nslot * F, tagp + "_u", dt=BF16)
            attb = att.rearrange("p (h f) -> p () h f", h=NH) \
                .to_broadcast([128, nslot, NH, C2])
            u4 = u[:].rearrange("p (s h f) -> p s h f", s=nslot, h=NH)
            tt(out=u4, in0=u4, in1=attb, op=OP.mult)
            lg = small.tile([128, nslot, NH], F32, tag=tagp + "_lg")
            red(out=lg[:], in_=u4, axis=AX.X, op=OP.add)
            act(out=lg[:], in_=lg[:], func=AF.Exp)
            mb = mask_ap.rearrange("p s -> p s ()") \
                .to_broadcast([128, nslot, NH])
            tt(out=lg[:], in0=lg[:], in1=mb, op=OP.mult)
            wb = lg[:].rearrange("p s h -> p s h ()") \
                .to_broadcast([128, nslot, NH, C2])
            xlg4 = xlg_tile[:].rearrange("p (s h f) -> p s h f", s=nslot,
                                         h=NH)
            tt(out=xlg4, in0=xlg4, in1=wb, op=OP.mult)
            return lg

        def softmax_combine(xlg_tile, lg, nslot, tagp):
            comb = small.tile([128, F + NH], F32, tag=tagp + "_comb")
            red(out=comb[:, 0:F],
                in_=xlg_tile[:].rearrange("p (s f) -> p f s", s=nslot),
                axis=AX.X, op=OP.add)
            red(out=comb[:, F:F + NH],
                in_=lg[:].rearrange("p s h -> p h s"),
                axis=AX.X, op=OP.add)
            return comb

        # ---------------- phase 1: base GAT layer 1 ----------------
        att1 = bfcast("att1")
        att2 = bfcast("att2")
        msk1 = bfcast("MSK1")
        mskbd = bfcast("MSKBD")
        g1b_chunks = []
        for h in range(2):
            xlg = xlg1[h]
            xrr = xrr1[h]
            tg = "p1h%d" % h
            xlg4 = xlg[:].rearrange("p (t s f) -> p t s f", t=4, s=S1)
            u = big.tile([128, 4 * S1 * F], BF16, tag=tg + "_u")
            u4 = u[:].rearrange("p (t s f) -> p t s f", t=4, s=S1)
            tt(out=u4, in0=xlg4,
               in1=xrr[:].rearrange("p t f -> p t () f")
               .to_broadcast([128, 4, S1, F]), op=OP.add)
            lrelu(u[:], 4 * S1 * F, tg + "_u", dt=BF16)
            attb = att1.rearrange("p (h f) -> p () () h f", h=NH) \
                .to_broadcast([128, 4, S1, NH, C2])
            u5 = u[:].rearrange("p (t s h f) -> p t s h f", t=4, s=S1,
                                h=NH)
            tt(out=u5, in0=u5, in1=attb, op=OP.mult)
            lg = small.tile([128, 4, S1, NH], F32, tag=tg + "_lg")
            red(out=lg[:], in_=u5, axis=AX.X, op=OP.add)
            act(out=lg[:], in_=lg[:], func=AF.Exp)
            msk = msk1[:, 4 * S1 * h:4 * S1 * (h + 1)] \
                .rearrange("p (t s) -> p t s", t=4)
            mb = msk.rearrange("p t s -> p t s ()") \
                .to_broadcast([128, 4, S1, NH])
            tt(out=lg[:], in0=lg[:], in1=mb, op=OP.mult)
            wb = lg[:].rearrange("p t s h -> p t s h ()") \
                .to_broadcast([128, 4, S1, NH, C2])
            xlg5 = xlg[:].rearrange("p (t s h f) -> p t s h f", t=4, s=S1,
                                    h=NH)
            tt(out=xlg5, in0=xlg5, in1=wb, op=OP.mult)

            comb = small.tile([128, 4, F + NH], F32, tag=tg + "_comb")
            red(out=comb[:, :, 0:F],
                in_=xlg[:].rearrange("p (t s f) -> p t f s", t=4, s=S1),
                axis=AX.X, op=OP.add)
            red(out=comb[:, :, F:F + NH],
                in_=lg[:].rearrange("p t s h -> p t h s"),
                axis=AX.X, op=OP.add)

            nd_ps = psum_acc.tile([128, F + NH], F32, tag=tg + "_ndps")
            cmb1 = cv("CMB1")
            for t in range(4):
                nc.tensor.matmul(nd_ps[:],
                                 cmb1[:, 128 * (4 * h + t):
                                      128 * (4 * h + t + 1)],
                                 comb[:, t, :],
                                 start=(t == 0), stop=(t == 3))
            nd = small.tile([128, F + NH], F32, tag=tg + "_nd")
            nc.vector.tensor_copy(out=nd[:], in_=nd_ps[:])
            dma(out=D["T1"][128 * h:128 * (h + 1), 0:F + NH], in_=nd[:])
            g1b = head_mean_bias_elu(nd[:], 128, cv("g1bias"), tg + "_g")
            g1b_chunks.append(g1b)

        # g1_base^T -> XL2_base (T2 rows 0:256)
        g1bT = small.tile([64, 256], F32, tag="g1bT")
        for h in range(2):
            ps = psum.tile([64, 128], F32, tag="ps")
            nc.tensor.transpose(ps[:], g1b_chunks[h], ident[:])
            nc.vector.tensor_copy(out=g1bT[:, 128 * h:128 * (h + 1)],
                                  in_=ps[:])
        for h in range(2):
            ps = psum.tile([128, 128], F32, tag="ps")
            nc.tensor.matmul(ps[:], g1bT[:, 128 * h:128 * (h + 1)],
                             cv("g2_wl"), start=True, stop=True)
            sb = small.tile([128, 128], BF16, tag="p15_sb%d" % h)
            nc.vector.tensor_copy(out=sb[:], in_=ps[:])
            dma(out=D["T2"][128 * h:128 * (h + 1), :], in_=sb[:])

        # ---------------- (b): full recompute of dst v ----------------
        lgb = edge_stage(xlgb, SB, mskbd[:], att1, xrrb, "b")
        combb = softmax_combine(xlgb, lgb, SB, "b")
        ndb_ps = psum.tile([VPC, F + NH], F32, tag="ps")
        nc.tensor.matmul(ndb_ps[:], cv("CMBBD"), combb[:],
                         start=True, stop=True)
        ndb = small.tile([VPC, F + NH], F32, tag="b_nd")
        nc.vector.tensor_copy(out=ndb[:], in_=ndb_ps[:])
        g1self = head_mean_bias_elu(ndb[:], VPC, cv("g1bias"), "bg1")

        ps_t = psum.tile([C2, VPC], F32, tag="ps")
        nc.tensor.transpose(ps_t[:], g1self, ident[:VPC, :VPC])
        g1sT = small.tile([C2, VPC], F32, tag="g1sT")
        nc.vector.tensor_copy(out=g1sT[:], in_=ps_t[:])
        ps_l = psum.tile([VPC, F], F32, tag="ps")
        nc.tensor.matmul(ps_l[:], g1sT[:], cv("g2_wl"), start=True, stop=True)
        sb_l = small.tile([VPC, F], BF16, tag="b_sbl")
        nc.vector.tensor_copy(out=sb_l[:], in_=ps_l[:])
        dma(out=D["T2"][N:N + VPC, :], in_=sb_l[:])
        ps_r = psum.tile([VPC, F], F32, tag="ps")
        nc.tensor.matmul(ps_r[:], g1sT[:], cv("g2_wr"), start=True, stop=True)
        sb_r = small.tile([VPC, F], BF16, tag="b_sbr")
        tt(out=sb_r[:], in0=ps_r[:], in1=cv("blr")[:VPC, :], op=OP.add)
        dma(out=D["XR2S"][:], in_=sb_r[:])

        # ---------------- (a): light dst updates ----------------
        t1nd = big.tile([128, KA * 192], F32, tag="a_t1nd")
        dgather(t1nd[:].rearrange("p (k f) -> p k f", k=KA),
                D["T1"][:], iv16("IDX_A_T1W"), KA * 128, 192)

        t1g3 = t1nd[:].rearrange("p (k f) -> p k f", k=KA)  # f = 192
        t1xr = t1xrg[:].rearrange("p (k f) -> p k f", k=KA)
        t1num = t1g3[:, :, 0:F]
        t1den = t1g3[:, :, F:F + NH]
        xlv3 = xlv[:].rearrange("p (k f) -> p k f", k=KA)
        xlsv3 = xlsv[:].rearrange("p (k f) -> p k f", k=KA)
        ca = cv("C_A")

        def logits_expC(xl3, tg):
            u = big.tile([128, KA * F], F32, tag="a_u" + tg)
            u3 = u[:].rearrange("p (k f) -> p k f", k=KA)
            tt(out=u3, in0=xl3, in1=t1xr, op=OP.add)
            lrelu(u[:], KA * F, "a_u" + tg)
            attb = cv("att1").rearrange("p (h f) -> p () h f", h=NH) \
                .to_broadcast([128, KA, NH, C2])
            u4 = u[:].rearrange("p (k h f) -> p k h f", k=KA, h=NH)
            tt(out=u4, in0=u4, in1=attb, op=OP.mult)
            lw = small.tile([128, KA, NH], F32, tag="a_lw" + tg)
            red(out=lw[:], in_=u4, axis=AX.X, op=OP.add)
            act(out=lw[:], in_=lw[:], func=AF.Exp)
            cb = ca.rearrange("p k -> p k ()").to_broadcast([128, KA, NH])
            tt(out=lw[:], in0=lw[:], in1=cb, op=OP.mult)
            return lw

        wn = logits_expC(xlsv3, "n")    # C * w_new
        wo = logits_expC(xlv3, "o")     # C * w_old

        dden = small.tile([128, KA, NH], F32, tag="a_dden")
        tt(out=dden[:], in0=wn[:], in1=wo[:], op=OP.subtract)
        tt(out=dden[:], in0=dden[:], in1=t1den, op=OP.add)
        dnum = big.tile([128, KA * F], F32, tag="a_dnum")
        dnum4 = dnum[:].rearrange("p (k h f) -> p k h f", k=KA, h=NH)
        dnum3 = dnum[:].rearrange("p (k f) -> p k f", k=KA)
        tmp = big.tile([128, KA * F], F32, tag="a_tmp")
        tmp4 = tmp[:].rearrange("p (k h f) -> p k h f", k=KA, h=NH)
        tmp3 = tmp[:].rearrange("p (k f) -> p k f", k=KA)
        wnb = wn[:].rearrange("p k h -> p k h ()") \
            .to_broadcast([128, KA, NH, C2])
        xlsv4 = xlsv[:].rearrange("p (k h f) -> p k h f", k=KA, h=NH)
        tt(out=dnum4, in0=xlsv4, in1=wnb, op=OP.mult)
        wob = wo[:].rearrange("p k h -> p k h ()") \
            .to_broadcast([128, KA, NH, C2])
        xlv4 = xlv[:].rearrange("p (k h f) -> p k h f", k=KA, h=NH)
        tt(out=tmp4, in0=xlv4, in1=wob, op=OP.mult)
        tt(out=dnum3, in0=dnum3, in1=tmp3, op=OP.subtract)
        tt(out=dnum3, in0=dnum3, in1=t1num, op=OP.add)
        nc.vector.reciprocal(out=dden[:], in_=dden[:])
        ddb = dden[:].rearrange("p k h -> p k h ()") \
            .to_broadcast([128, KA, NH, C2])
        tt(out=dnum4, in0=dnum4, in1=ddb, op=OP.mult)
        radd = big.tile([128, KA, C2], F32, tag="a_radd")
        tt(out=radd[:], in0=dnum4[:, :, 0, :], in1=dnum4[:, :, 1, :],
           op=OP.add)
        ts_mul(radd[:], radd[:], 0.5)
        g1bb = cv("g1bias").rearrange("p f -> p () f").to_broadcast(
            [128, KA, C2])
        tt(out=radd[:], in0=radd[:], in1=g1bb, op=OP.add)
        radd_flat = radd[:].rearrange("p k f -> p (k f)")
        elu_inplace(radd_flat, big, KA * C2, "a_elu")
        dma(out=D["G1L"][:].rearrange("(p k) f -> p k f", p=128), in_=radd[:])

        # ---------------- rare light rows -> T2 rows 288: ----------------
        grare = small.tile([128, C2], F32, tag="r_g")
        gather(out=grare[:], out_offset=None, in_=D["G1L"][:],
               in_offset=IOA(ap=iv32("IDX_RARE"), axis=0))
        ps_rt = psum.tile([C2, 128], F32, tag="ps")
        nc.tensor.transpose(ps_rt[:], grare[:], ident[:])
        grT = small.tile([C2, 128], F32, tag="grT")
        nc.vector.tensor_copy(out=grT[:], in_=ps_rt[:])
        ps_rm = psum.tile([128, F], F32, tag="ps")
        nc.tensor.matmul(ps_rm[:], grT[:], cv("g2_wl"), start=True, stop=True)
        sb_rm = small.tile([128, F], BF16, tag="r_sb")
        nc.vector.tensor_copy(out=sb_rm[:], in_=ps_rm[:])
        dma(out=D["T2"][N + VPC:N + VPC + 128, :], in_=sb_rm[:])

        # ---------------- (d): layer 2 at dst v ----------------
        xl2g = big.tile([128, SB * F], BF16, tag="d_xlg")
        dgather(xl2g[:].rearrange("p (k f) -> p k f", k=SB),
                D["T2"][:], iv16("IDX_DW"), SB * 128, F)
        xr2r = big.tile([128, F], BF16, tag="d_xrr")
        gather(out=xr2r[:], out_offset=None, in_=D["XR2S"][:],
               in_offset=IOA(ap=iv32("IDX_D_V"), axis=0))
        lgd = edge_stage(xl2g, SB, mskbd[:], att2, xr2r, "d")
        combd = softmax_combine(xl2g, lgd, SB, "d")
        ndd_ps = psum.tile([VPC, F + NH], F32, tag="ps")
        nc.tensor.matmul(ndd_ps[:], cv("CMBBD"), combd[:],
                         start=True, stop=True)
        ndd = small.tile([VPC, F + NH], F32, tag="d_nd")
        nc.vector.tensor_copy(out=ndd[:], in_=ndd_ps[:])
        g2row = head_mean_bias_elu(ndd[:], VPC, cv("g2bias"), "dg2")

        # out = tanh(g2row @ rec_w + rec_b)
        ps_ot = psum.tile([C2, VPC], F32, tag="ps")
        nc.tensor.transpose(ps_ot[:], g2row, ident[:VPC, :VPC])
        g2T = small.tile([C2, VPC], F32, tag="g2T")
        nc.vector.tensor_copy(out=g2T[:], in_=ps_ot[:])
        ps_om = psum.tile([C2, VPC], F32, tag="ps")
        nc.tensor.matmul(ps_om[:], cv("rec_w"), g2T[:], start=True,
                         stop=True)
        outT = small.tile([C2, VPC], F32, tag="outT")
        act(out=outT[:], in_=ps_om[:], func=AF.Tanh, bias=cv("rec_b"))
        ps_of = psum.tile([VPC, C2], F32, tag="ps")
        nc.tensor.transpose(ps_of[:], outT[:], ident[:C2, :C2])
        outsb = small.tile([VPC, C2], F32, tag="outsb")
        nc.vector.tensor_copy(out=outsb[:], in_=ps_of[:])
        dma(out=D["out"][:], in_=outsb[:])


# --------------------------------------------------------------------------
# Entry point
# --------------------------------------------------------------------------

def _make_in_maps(inputs, shared, percore, dims):
    f32 = np.float32
    (off_f, nf), (off_i16, ni16), (off_i32, ni32) = _pack_layout(dims)

    def rep(v):
        a = np.asarray(v, f32).reshape(1, -1)
        return np.ascontiguousarray(np.broadcast_to(a, (128, a.shape[1])))

    vals = {
        "conv_b": np.asarray(inputs["conv_b"], f32).reshape(128, 1),
        "lin2_b": np.asarray(inputs["lin2_b"], f32).reshape(64, 1),
        "g1_bl": np.asarray(inputs["g1_bl"], f32).reshape(128, 1),
        "g1_br": np.asarray(inputs["g1_br"], f32).reshape(128, 1),
        "rec_b": np.asarray(inputs["rec_b"], f32).reshape(64, 1),
        "att1": rep(inputs["g1_att"]),
        "att2": rep(inputs["g2_att"]),
        "g1bias": rep(inputs["g1_bias"]),
        "g2bias": rep(inputs["g2_bias"]),
        "blr": rep(inputs["g2_bl"] + inputs["g2_br"]),
        "CMB1": shared["CMB1"].transpose(1, 0, 2, 3).reshape(128, -1),
        "MSK1": shared["MSK1"].transpose(1, 0, 2).reshape(128, -1),
    }
    for nm in ("node_proj", "emb_proj", "conv_w0", "conv_w1", "lin2_w",
               "masked_proj", "normal_proj", "g1_wl", "g1_wr", "g2_wl",
               "g2_wr", "rec_w"):
        vals[nm] = np.asarray(inputs[nm], f32)

    x = np.asarray(inputs["x"], f32)
    E = np.asarray(inputs["E_emb"], f32)
    xE = np.concatenate([x[0:128], x[128:256], E[0:128], E[128:256]],
                        axis=1)

    def fill(off_map, total, npdtype, core_vals):
        out = np.zeros((128, total), npdtype)
        for nm, (o, rows, cols) in off_map.items():
            a = core_vals[nm]
            assert a.shape == (rows, cols) or a.shape[0] <= rows, \
                (nm, a.shape, rows, cols)
            out[:a.shape[0], o:o + cols] = a
        return out

    in_maps = []
    for c in range(NCORES):
        t = percore[c]
        cvals = dict(vals)
        cvals["CMBBD"] = t["CMBBD"]
        cvals["MSKBD"] = t["MSKBD"]
        cvals["C_A"] = t["C_A"]
        i16vals = {"IDX1W0": shared["IDX1W"][0],
                   "IDX1W1": shared["IDX1W"][1],
                   "IDX_BW": t["IDX_BW"], "IDX_DW": t["IDX_DW"],
                   "IDX_A_T1W": t["IDX_A_T1W"], "IDX_A_VW": t["IDX_A_VW"],
                   "IDX_A_VSW": t["IDX_A_VSW"]}
        i32vals = {"IDXD1": shared["IDXD1"].transpose(1, 0, 2)
                   .reshape(128, 8),
                   "IDX_B_V": t["IDX_B_V"], "IDX_D_V": t["IDX_D_V"],
                   "IDX_RARE": t["IDX_RARE"]}
        in_maps.append({
            "packf": fill(off_f, nf, np.float32, cvals),
            "packi16": fill(off_i16, ni16, np.int16, i16vals),
            "packi32": fill(off_i32, ni32, np.int32, i32vals),
            "xE": np.ascontiguousarray(xE),
        })
    return in_maps


_CACHE = {}
TRACE = False          # set by test.py to capture NTFF profiles
LAST_RESULT = None


def kernel(**inputs):
    global LAST_RESULT
    inputs = {k: np.asarray(v) for k, v in inputs.items()}
    shared, percore, dims = _build_tables(inputs["edge_index"])
    key = (dims["S1"], dims["SB"], dims["K2"])
    if key not in _CACHE:
        _CACHE[key] = _build_program(dims)
    nc = _CACHE[key]
    in_maps = _make_in_maps(inputs, shared, percore, dims)
    kw = {}
    if TRACE:
        kw = dict(trace=True, trace_cores=list(range(NCORES)))
    res = run_bass_kernel_spmd(nc, in_maps, core_ids=list(range(NCORES)),
                               **kw)
    LAST_RESULT = res
    out = np.concatenate([res.results[c]["out"] for c in range(NCORES)],
                         axis=0)
    return out.astype(np.float32)
